# revision 1
# baseline (speedup 1.0000x reference)
"""Trainium2 Bass kernel for the CurvedAssociativeMemory fixed-point iteration.

Computes, for `steps` iterations:
    s <- sign(s @ (J + J^T) + h + kappa * softmax(s, axis=-1))

Strategy: data-parallel over the batch dim across 8 NeuronCores (512 rows
per core), J replicated and streamed from HBM each step.  All matmuls are
native fp32 with K accumulated in ascending 128-row chunks in PSUM, which
bit-matches the XLA lowering of the jax reference on this hardware.  The
softmax is computed in the natural layout with the same op sequence XLA
emits (max-subtract, ACT-table exp, free-dim reduce_sum, DVE reciprocal +
multiply), so the full pipeline tracks the reference to within a few ulps.
"""

import numpy as np

N = 4096          # feature dim
B = 4096          # total batch
N_CORES = 8
B_SH = B // N_CORES   # 512 batch rows per core
P = 128               # partitions
NCHUNK = 256          # matmul moving free-dim per chunk
KO = N // P           # 32 k-tiles
NO = N // NCHUNK      # 8 n-chunks
BT = B_SH // P        # 4 batch tiles per core

# tuning knobs (overridable before _build for experiments)
REPEAT = 1  # timing only: run the whole step body REPEAT times via a HW loop
JPOOL_BUFS = 4
SCRATCH_BUFS = 2
PSUM_BUFS = 8


def _build(steps: int, kappa: float, has_h: bool):
    NCHUNK_ = NCHUNK; NO_ = N // NCHUNK_
    import concourse.bass as bass
    import concourse.tile as tile
    import concourse.mybir as mybir
    from concourse import bacc
    from concourse.masks import make_identity

    F32 = mybir.dt.float32
    AF = mybir.ActivationFunctionType

    nc = bacc.Bacc(None)
    s_in = nc.dram_tensor("s", [B_SH, N], F32, kind="ExternalInput")
    j_in = nc.dram_tensor("J", [N, N], F32, kind="ExternalInput")
    h_in = nc.dram_tensor("h", [N], F32, kind="ExternalInput") if has_h else None
    out = nc.dram_tensor("out", [B_SH, N], F32, kind="ExternalOutput")

    with tile.TileContext(nc) as tc:
        with (
            tc.tile_pool(name="persist", bufs=1) as persist,
            tc.tile_pool(name="jpool", bufs=JPOOL_BUFS) as jpool,
            tc.tile_pool(name="scratch", bufs=SCRATCH_BUFS) as scratch,
            tc.tile_pool(name="stats", bufs=1) as stats,
            tc.tile_pool(name="psum", bufs=PSUM_BUFS, space="PSUM") as psum,
        ):
            ident = persist.tile([P, P], F32, tag="ident", name="ident")
            make_identity(nc, ident)

            # persistent state: c in natural layout, 4 tiles of [128, N]
            c = [persist.tile([P, N], F32, tag=f"c{bt}", name=f"c{bt}") for bt in range(BT)]
            for bt in range(BT):
                nc.sync.dma_start(out=c[bt], in_=s_in.ap()[bt * P:(bt + 1) * P, :])

            # transposed state: cT, 32 tiles of [128, B_SH]
            cT = [persist.tile([P, B_SH], F32, tag=f"t{k}", name=f"t{k}") for k in range(KO)]

            h_bc = None
            if has_h:
                h_bc = persist.tile([P, N], F32, tag="hb", name="hb")
                h_ap = h_in.ap()
                nc.sync.dma_start(
                    out=h_bc,
                    in_=bass.AP(tensor=h_ap.tensor, offset=h_ap.offset,
                                ap=[[0, P], [1, N]]),
                )

            mx = [stats.tile([P, 1], F32, tag=f"mx{bt}", name=f"mx{bt}") for bt in range(BT)]
            rS = [stats.tile([P, 1], F32, tag=f"rS{bt}", name=f"rS{bt}") for bt in range(BT)]

            def emit_steps():
                for _step in range(steps):
                    # ---- phase A: transpose c -> cT, softmax stats per b-tile ----
                    # k-major so cT[k] completes early and the k=0 matmuls can
                    # start while later k-tiles are still transposing.
                    for k in range(KO):
                        for bt in range(BT):
                            ps_t = psum.tile([P, NCHUNK_], F32, tag="pb", name="ps_t")[:, :P]
                            nc.tensor.transpose(ps_t, c[bt][:, k * P:(k + 1) * P], ident)
                            nc.vector.tensor_copy(
                                out=cT[k][:, bt * P:(bt + 1) * P], in_=ps_t)

                    for bt in range(BT):
                        et = scratch.tile([P, N], F32, tag="et", name="et")
                        nc.vector.reduce_max(out=mx[bt], in_=c[bt],
                                             axis=mybir.AxisListType.X)
                        nc.vector.tensor_scalar_sub(out=et, in0=c[bt], scalar1=mx[bt])
                        nc.scalar.activation(out=et, in_=et, func=AF.Exp)
                        ssum = stats.tile([P, 1], F32, tag="ssum", name="ssum")
                        nc.vector.reduce_sum(out=ssum, in_=et,
                                             axis=mybir.AxisListType.X)
                        nc.vector.reciprocal(out=rS[bt], in_=ssum)

                    # ---- phase B: matmul + epilogue per n-chunk ----
                    for n in range(NO_):
                        pm_t = [psum.tile([P, NCHUNK_], F32, tag="pb", name="pm")
                                for _ in range(BT)]
                        for k in range(KO):
                            jt = jpool.tile([P, NCHUNK_], F32, tag="jt", name="jt")
                            nc.sync.dma_start(
                                out=jt,
                                in_=j_in.ap()[k * P:(k + 1) * P,
                                              n * NCHUNK_:(n + 1) * NCHUNK_])
                            for bt in range(BT):
                                nc.tensor.matmul(
                                    pm_t[bt],
                                    cT[k][:, bt * P:(bt + 1) * P],
                                    jt,
                                    start=(k == 0), stop=(k == KO - 1))
                        nsl = slice(n * NCHUNK_, (n + 1) * NCHUNK_)
                        for bt in range(BT):
                            m_sl = pm_t[bt]
                            u = scratch.tile([P, NCHUNK_], F32, tag="u", name="u")
                            if has_h:
                                nc.vector.tensor_add(out=u, in0=m_sl, in1=h_bc[:, nsl])
                            q = scratch.tile([P, NCHUNK_], F32, tag="q", name="q")
                            nc.vector.tensor_scalar_sub(out=q, in0=c[bt][:, nsl],
                                                        scalar1=mx[bt])
                            nc.scalar.activation(out=q, in_=q, func=AF.Exp)
                            nc.vector.tensor_scalar_mul(out=q, in0=q, scalar1=rS[bt])
                            nc.scalar.mul(out=q, in_=q, mul=float(kappa))
                            if has_h:
                                nc.vector.tensor_add(out=u, in0=u, in1=q)
                            else:
                                nc.vector.tensor_add(out=u, in0=m_sl, in1=q)
                            nc.scalar.activation(out=c[bt][:, nsl], in_=u, func=AF.Sign)


            if REPEAT > 1:
                with tc.For_i(0, REPEAT, 1):
                    emit_steps()
            else:
                emit_steps()

            for bt in range(BT):
                nc.sync.dma_start(out=out.ap()[bt * P:(bt + 1) * P, :], in_=c[bt])

    nc.finalize()
    return nc


LAST_RESULTS = None  # BassKernelResults from the most recent kernel() call
LAST_NC = None       # finalized Bass module from the most recent kernel() call


def kernel(s, J, h, kappa, steps):
    import os
    from concourse.bass_utils import run_bass_kernel_spmd

    s = np.ascontiguousarray(np.asarray(s, dtype=np.float32))
    J = np.asarray(J, dtype=np.float32)
    h = np.asarray(h, dtype=np.float32)
    kappa_f = float(np.asarray(kappa))
    steps_i = int(np.asarray(steps))

    Jsym = np.ascontiguousarray(J + J.T)
    has_h = bool(np.any(h))

    nc = _build(steps_i, kappa_f, has_h)
    global LAST_NC
    LAST_NC = nc

    in_maps = []
    for i in range(N_CORES):
        m = {"s": np.ascontiguousarray(s[i * B_SH:(i + 1) * B_SH]), "J": Jsym}
        if has_h:
            m["h"] = h
        in_maps.append(m)

    trace = os.environ.get("CAM_TRACE", "") == "1"
    res = run_bass_kernel_spmd(nc, in_maps, core_ids=list(range(N_CORES)),
                               trace=trace)
    global LAST_RESULTS
    LAST_RESULTS = res
    out = np.concatenate([r["out"] for r in res.results], axis=0)
    return out.astype(np.float32, copy=False)


if __name__ == "__main__":
    rng = np.random.default_rng(0)
    s = rng.standard_normal((B, N)).astype(np.float32)
    J0 = (0.01 * rng.standard_normal((N, N))).astype(np.float32)
    J = ((J0 + J0.T) / 2).astype(np.float32)
    out = kernel(s=s, J=J, h=np.zeros(N, np.float32),
                 kappa=np.float32(0.2), steps=3)
    print(out.shape, np.unique(out, return_counts=True))



# revision 6
# speedup vs baseline: 1.3379x; 1.3379x over previous
"""Trainium2 Bass kernel for the CurvedAssociativeMemory fixed-point iteration.

Computes, for `steps` iterations:
    s <- sign(s @ (J + J^T) + h + kappa * softmax(s, axis=-1))

Strategy: data-parallel over batch across 8 NeuronCores (512 rows/core),
J replicated and streamed from HBM each step.

Step 1 is native fp32 (4 PE passes/matmul) with K accumulated in ascending
128-row chunks in PSUM — this bit-matches the XLA lowering of the jax
reference on this hardware, which matters because sign() flips amplify
~90x through the remaining steps.

Steps 2..n exploit that the state is exactly {-1,+1} after step 1:
  * the matmul runs as TWO bf16 passes against a J = J_hi + J_lo split
    (round-to-nearest bf16 hi + bf16 residual).  s is exact in bf16, so
    the only deviation from the device's native fp32 decomposition is
    J's representation error (~2^-18), measured end-to-end at ~250
    flipped signs out of 16.7M (rel err ~8e-3, gate is 2e-2).  4x fewer
    PE cycles than native fp32.
  * softmax(c) for c in {-1,+1} is linear in c:
    kappa*softmax(c) = A[b] + B[b]*c with A = kappa*cosh(1)/D,
    B = kappa*sinh(1)/D, D = 4096*cosh(1) + sinh(1)*sum(c) - so no exp,
    no max-reduce, just a row-sum and two fused scalar ops.
"""

import math

import numpy as np

N = 4096          # feature dim
B = 4096          # total batch
N_CORES = 8
B_SH = B // N_CORES   # 512 batch rows per core
P = 128               # partitions
NCHUNK = 512          # matmul moving free-dim per chunk
KO = N // P           # 32 k-tiles
NO = N // NCHUNK      # 8 n-chunks
BT = B_SH // P        # 4 batch tiles per core

JPOOL_BUFS = 6
PSUM_BUFS = 8


def _build(steps: int, kappa: float, has_h: bool):
    import concourse.bass as bass
    import concourse.tile as tile
    import concourse.mybir as mybir
    from concourse import bacc
    from concourse.masks import make_identity

    F32 = mybir.dt.float32
    BF16 = mybir.dt.bfloat16
    AF = mybir.ActivationFunctionType
    ALU = mybir.AluOpType
    X = mybir.AxisListType.X

    kcosh = float(kappa * math.cosh(1.0))
    ksinh = float(kappa * math.sinh(1.0))
    dconst = float(N * math.cosh(1.0))
    sinh1 = float(math.sinh(1.0))

    nc = bacc.Bacc(None)
    s_in = nc.dram_tensor("s", [B_SH, N], F32, kind="ExternalInput")
    j_in = nc.dram_tensor("J", [N, N], F32, kind="ExternalInput")
    jh_in = nc.dram_tensor("JH", [N, N], BF16, kind="ExternalInput")
    jl_in = nc.dram_tensor("JL", [N, N], BF16, kind="ExternalInput")
    h_in = nc.dram_tensor("h", [N], F32, kind="ExternalInput") if has_h else None
    out = nc.dram_tensor("out", [B_SH, N], F32, kind="ExternalOutput")

    with tile.TileContext(nc) as tc:
        with (
            tc.tile_pool(name="persist", bufs=1) as persist,
            tc.tile_pool(name="stats", bufs=1) as stats,
            tc.tile_pool(name="scratch", bufs=4) as scratch,
            tc.tile_pool(name="psum", bufs=6, space="PSUM") as psum,
            tc.tile_pool(name="psumt", bufs=2, space="PSUM") as psumt,
        ):
            identf = persist.tile([P, P], F32, tag="identf", name="identf")
            make_identity(nc, identf)
            identb = persist.tile([P, P], BF16, tag="identb", name="identb")
            make_identity(nc, identb)

            h_bc = None
            if has_h:
                h_bc = persist.tile([P, N], F32, tag="hb", name="hb")
                h_ap = h_in.ap()
                nc.sync.dma_start(
                    out=h_bc,
                    in_=bass.AP(tensor=h_ap.tensor, offset=h_ap.offset,
                                ap=[[0, P], [1, N]]),
                )

            # steps>=2 state: sign values as bf16 (exact)
            cb = [persist.tile([P, N], BF16, tag=f"cb{bt}", name=f"cb{bt}")
                  for bt in range(BT)]
            cTb = [persist.tile([P, B_SH], BF16, tag=f"tb{k}", name=f"tb{k}")
                   for k in range(KO)]

            # ---------------- STEP 1: native fp32, bit-exact ----------------
            # s is streamed from HBM per use (transpose blocks, stats chunks,
            # epilogue chunks) instead of held resident: frees 8MB of SBUF for
            # ~24MB of extra, fully-hidden DMA.
            with (
                tc.tile_pool(name="s1", bufs=1) as s1,
                tc.tile_pool(name="jpool1", bufs=JPOOL_BUFS) as jpool1,
                tc.tile_pool(name="spool", bufs=4) as spool,
            ):
                cT = [s1.tile([P, B_SH], F32, tag=f"t{k}", name=f"t{k}")
                      for k in range(KO)]

                # transpose s -> cT, k-major so k=0 matmuls start early
                for k in range(KO):
                    for bt in range(BT):
                        ck = spool.tile([P, P], F32, tag="ck", name="ck")
                        nc.sync.dma_start(
                            out=ck, in_=s_in.ap()[bt * P:(bt + 1) * P,
                                                  k * P:(k + 1) * P])
                        ps_t = psum.tile([P, NCHUNK], F32, tag="pb",
                                         name="ps_t")[:, :P]
                        nc.tensor.transpose(ps_t, ck, identf)
                        nc.vector.tensor_copy(
                            out=cT[k][:, bt * P:(bt + 1) * P], in_=ps_t)

                # softmax stats, chunked (max is order-exact; the exp-sum D
                # only needs ~1e-4 relative accuracy for sign() stability)
                mx = [stats.tile([P, 1], F32, tag=f"mx{bt}", name=f"mx{bt}")
                      for bt in range(BT)]
                rS = [stats.tile([P, 1], F32, tag=f"rS{bt}", name=f"rS{bt}")
                      for bt in range(BT)]
                for bt in range(BT):
                    cmx = [stats.tile([P, 1], F32, tag=f"cm{bt}_{i}",
                                      name=f"cm{bt}_{i}") for i in range(NO)]
                    for nch in range(NO):
                        nsl = slice(nch * NCHUNK, (nch + 1) * NCHUNK)
                        cc = spool.tile([P, NCHUNK], F32, tag="cc", name="cc")
                        nc.sync.dma_start(
                            out=cc, in_=s_in.ap()[bt * P:(bt + 1) * P, nsl])
                        nc.vector.reduce_max(out=cmx[nch], in_=cc, axis=X)
                    for nch in range(1, NO):
                        nc.vector.tensor_max(out=cmx[0], in0=cmx[0],
                                             in1=cmx[nch])
                    nc.vector.tensor_copy(out=mx[bt], in_=cmx[0])
                    ssum = stats.tile([P, 1], F32, tag=f"ss{bt}", name=f"ss{bt}")
                    for nch in range(NO):
                        nsl = slice(nch * NCHUNK, (nch + 1) * NCHUNK)
                        cc = spool.tile([P, NCHUNK], F32, tag="cc", name="cc")
                        nc.sync.dma_start(
                            out=cc, in_=s_in.ap()[bt * P:(bt + 1) * P, nsl])
                        et = scratch.tile([P, NCHUNK], F32, tag="q", name="et")
                        nc.vector.tensor_scalar_sub(out=et, in0=cc,
                                                    scalar1=mx[bt])
                        nc.scalar.activation(out=et, in_=et, func=AF.Exp)
                        pk = stats.tile([P, 1], F32, tag=f"pk{bt}_{nch}",
                                        name=f"pk{bt}_{nch}")
                        nc.vector.reduce_sum(out=pk, in_=et, axis=X)
                        if nch == 0:
                            nc.vector.tensor_copy(out=ssum, in_=pk)
                        else:
                            nc.vector.tensor_add(out=ssum, in0=ssum, in1=pk)
                    nc.vector.reciprocal(out=rS[bt], in_=ssum)

                # matmul + epilogue per n-chunk
                for n in range(NO):
                    nsl = slice(n * NCHUNK, (n + 1) * NCHUNK)
                    pm_t = [psum.tile([P, NCHUNK], F32, tag="pb", name="pm")
                            for _ in range(BT)]
                    for k in range(KO):
                        jt = jpool1.tile([P, NCHUNK], F32, tag="jt", name="jt")
                        nc.sync.dma_start(
                            out=jt, in_=j_in.ap()[k * P:(k + 1) * P, nsl])
                        for bt in range(BT):
                            nc.tensor.matmul(
                                pm_t[bt],
                                cT[k][:, bt * P:(bt + 1) * P],
                                jt,
                                start=(k == 0), stop=(k == KO - 1))
                    for bt in range(BT):
                        m_sl = pm_t[bt]
                        cc = spool.tile([P, NCHUNK], F32, tag="cc", name="cc")
                        nc.sync.dma_start(
                            out=cc, in_=s_in.ap()[bt * P:(bt + 1) * P, nsl])
                        u = None
                        if has_h:
                            u = scratch.tile([P, NCHUNK], F32, tag="u", name="u")
                            nc.vector.tensor_add(out=u, in0=m_sl,
                                                 in1=h_bc[:, nsl])
                        q = scratch.tile([P, NCHUNK], F32, tag="q", name="q")
                        nc.vector.tensor_scalar_sub(out=q, in0=cc,
                                                    scalar1=mx[bt])
                        nc.scalar.activation(out=q, in_=q, func=AF.Exp)
                        nc.vector.tensor_scalar_mul(out=q, in0=q, scalar1=rS[bt])
                        nc.scalar.mul(out=q, in_=q, mul=float(kappa))
                        uu = scratch.tile([P, NCHUNK], F32, tag="uu", name="uu")
                        if has_h:
                            nc.vector.tensor_add(out=uu, in0=u, in1=q)
                        else:
                            nc.vector.tensor_add(out=uu, in0=m_sl, in1=q)
                        if steps == 1:
                            ot = scratch.tile([P, NCHUNK], F32, tag="ot",
                                              name="ot")
                            nc.scalar.activation(out=ot, in_=uu, func=AF.Sign)
                            nc.sync.dma_start(
                                out=out.ap()[bt * P:(bt + 1) * P, nsl], in_=ot)
                        else:
                            nc.scalar.activation(out=cb[bt][:, nsl], in_=uu,
                                                 func=AF.Sign)

            # ---------------- STEPS 2..n: bf16 2-pass ----------------
            with tc.tile_pool(name="jpool2", bufs=JPOOL_BUFS) as jpool2:
                for si in range(1, steps):
                    last = (si == steps - 1)

                    # transpose cb -> cTb (bf16, values +-1 exact)
                    for k in range(KO):
                        for bt in range(BT):
                            ps_t = psumt.tile([P, 2 * NCHUNK], BF16, tag="ptb",
                                              name="ps_t")[:, :P]
                            nc.tensor.transpose(
                                ps_t, cb[bt][:, k * P:(k + 1) * P], identb)
                            nc.vector.tensor_copy(
                                out=cTb[k][:, bt * P:(bt + 1) * P], in_=ps_t)

                    # linearized softmax coefficients
                    A_ap = [stats.tile([P, 1], F32, tag=f"A{bt}", name=f"A{bt}")
                            for bt in range(BT)]
                    B_ap = [stats.tile([P, 1], F32, tag=f"B{bt}", name=f"B{bt}")
                            for bt in range(BT)]
                    for bt in range(BT):
                        S = stats.tile([P, 1], F32, tag=f"S{bt}", name=f"S{bt}")
                        nc.vector.reduce_sum(out=S, in_=cb[bt], axis=X)
                        D = stats.tile([P, 1], F32, tag=f"D{bt}", name=f"D{bt}")
                        nc.vector.tensor_scalar(out=D, in0=S, scalar1=sinh1,
                                                scalar2=dconst, op0=ALU.mult,
                                                op1=ALU.add)
                        rec = stats.tile([P, 1], F32, tag=f"rc{bt}",
                                         name=f"rc{bt}")
                        nc.vector.reciprocal(out=rec, in_=D)
                        nc.vector.tensor_scalar_mul(out=A_ap[bt], in0=rec,
                                                    scalar1=kcosh)
                        nc.vector.tensor_scalar_mul(out=B_ap[bt], in0=rec,
                                                    scalar1=ksinh)

                    for n in range(NO):
                        nsl = slice(n * NCHUNK, (n + 1) * NCHUNK)
                        pm_t = [psum.tile([P, NCHUNK], F32, tag="pb", name="pm")
                                for _ in range(BT)]
                        for k in range(KO):
                            jh = jpool2.tile([P, NCHUNK], BF16, tag="jb",
                                             name="jh")
                            nc.sync.dma_start(
                                out=jh, in_=jh_in.ap()[k * P:(k + 1) * P, nsl])
                            jl = jpool2.tile([P, NCHUNK], BF16, tag="jb",
                                             name="jl")
                            nc.sync.dma_start(
                                out=jl, in_=jl_in.ap()[k * P:(k + 1) * P, nsl])
                            for bt in range(BT):
                                sl = cTb[k][:, bt * P:(bt + 1) * P]
                                nc.tensor.matmul(pm_t[bt], sl, jh,
                                                 start=(k == 0), stop=False)
                                nc.tensor.matmul(pm_t[bt], sl, jl,
                                                 start=False, stop=(k == KO - 1))
                        for bt in range(BT):
                            # u = cb*B + mm;  sign(u + A)  — together these add
                            # kappa*softmax(cb) (exact linearization for +-1)
                            u = scratch.tile([P, NCHUNK], F32, tag="uu",
                                             name="u")
                            nc.vector.scalar_tensor_tensor(
                                out=u, in0=cb[bt][:, nsl], scalar=B_ap[bt],
                                in1=pm_t[bt], op0=ALU.mult, op1=ALU.add)
                            if has_h:
                                nc.vector.tensor_add(out=u, in0=u,
                                                     in1=h_bc[:, nsl])
                            if last:
                                ot = scratch.tile([P, NCHUNK], F32, tag="ot",
                                                  name="ot")
                                nc.scalar.sign(ot, u, bias=A_ap[bt])
                                nc.sync.dma_start(
                                    out=out.ap()[bt * P:(bt + 1) * P, nsl],
                                    in_=ot)
                            else:
                                nc.scalar.sign(cb[bt][:, nsl], u,
                                               bias=A_ap[bt])

    nc.finalize()
    return nc


LAST_RESULTS = None  # BassKernelResults from the most recent kernel() call
LAST_NC = None       # finalized Bass module from the most recent kernel() call


def kernel(s, J, h, kappa, steps):
    import os
    import ml_dtypes
    from concourse.bass_utils import run_bass_kernel_spmd

    s = np.ascontiguousarray(np.asarray(s, dtype=np.float32))
    J = np.asarray(J, dtype=np.float32)
    h = np.asarray(h, dtype=np.float32)
    kappa_f = float(np.asarray(kappa))
    steps_i = int(np.asarray(steps))

    Jsym = np.ascontiguousarray(J + J.T)
    JH = Jsym.astype(ml_dtypes.bfloat16)
    JL = (Jsym - JH.astype(np.float32)).astype(ml_dtypes.bfloat16)
    JH = np.ascontiguousarray(JH)
    JL = np.ascontiguousarray(JL)
    has_h = bool(np.any(h))

    nc = _build(steps_i, kappa_f, has_h)
    global LAST_NC
    LAST_NC = nc

    in_maps = []
    for i in range(N_CORES):
        m = {"s": np.ascontiguousarray(s[i * B_SH:(i + 1) * B_SH]),
             "J": Jsym, "JH": JH, "JL": JL}
        if has_h:
            m["h"] = h
        in_maps.append(m)

    trace = os.environ.get("CAM_TRACE", "") == "1"
    res = run_bass_kernel_spmd(nc, in_maps, core_ids=list(range(N_CORES)),
                               trace=trace)
    global LAST_RESULTS
    LAST_RESULTS = res
    out = np.concatenate([r["out"] for r in res.results], axis=0)
    return out.astype(np.float32, copy=False)


if __name__ == "__main__":
    rng = np.random.default_rng(0)
    s = rng.standard_normal((B, N)).astype(np.float32)
    J0 = (0.01 * rng.standard_normal((N, N))).astype(np.float32)
    J = ((J0 + J0.T) / 2).astype(np.float32)
    out = kernel(s=s, J=J, h=np.zeros(N, np.float32),
                 kappa=np.float32(0.2), steps=3)
    print(out.shape, np.unique(out, return_counts=True))


# revision 7
# speedup vs baseline: 1.3969x; 1.0441x over previous
"""Trainium2 Bass kernel for the CurvedAssociativeMemory fixed-point iteration.

Computes, for `steps` iterations:
    s <- sign(s @ (J + J^T) + h + kappa * softmax(s, axis=-1))

Strategy: data-parallel over batch across 8 NeuronCores (512 rows/core),
J replicated and streamed from HBM each step.

Step 1 is native fp32 (2 HW passes per matmul, H/L split of the fp32
operands) with K accumulated in ascending 128-row chunks in PSUM - this
bit-matches the XLA lowering of the jax reference on this hardware, which
matters because sign() flips amplify ~90x through the remaining steps.

Steps 2..n exploit that the state is exactly {-1,+1} after step 1:
  * the matmul runs as TWO bf16 passes against a J = J_hi + J_lo split
    (round-to-nearest bf16 hi + bf16 residual).  s is exact in bf16, so
    the only deviation from the device's native fp32 matmul is J's
    representation error (~2^-18), measured end-to-end at ~250 flipped
    signs out of 16.7M (rel err ~8e-3, gate 2e-2).  2x fewer PE cycles
    than native fp32 (1 cyc/col bf16 streaming vs 2 cyc/col fp32).
  * softmax(c) for c in {-1,+1} is linear in c:
    kappa*softmax(c) = A[b] + B[b]*c with A = kappa*cosh(1)/D,
    B = kappa*sinh(1)/D, D = 4096*cosh(1) + sinh(1)*sum(c) - no exp and
    no max-reduce, just a row-sum and two fused ops.

The transpose of the state (producing the stationary operand) is folded
into the n=0 chunk of each step's matmul loop so the tensor engine never
drains, and J-tile DMAs are not queued behind bulk state DMAs.
"""

import math

import numpy as np

N = 4096          # feature dim
B = 4096          # total batch
N_CORES = 8
B_SH = B // N_CORES   # 512 batch rows per core
P = 128               # partitions
NCHUNK = 512          # matmul moving free-dim per chunk
KO = N // P           # 32 k-tiles
NO = N // NCHUNK      # 8 n-chunks
BT = B_SH // P        # 4 batch tiles per core

JPOOL_BUFS = 6


def _build(steps: int, kappa: float, has_h: bool):
    import concourse.bass as bass
    import concourse.tile as tile
    import concourse.mybir as mybir
    from concourse import bacc
    from concourse.masks import make_identity

    F32 = mybir.dt.float32
    BF16 = mybir.dt.bfloat16
    AF = mybir.ActivationFunctionType
    ALU = mybir.AluOpType
    X = mybir.AxisListType.X

    kcosh = float(kappa * math.cosh(1.0))
    ksinh = float(kappa * math.sinh(1.0))
    dconst = float(N * math.cosh(1.0))
    sinh1 = float(math.sinh(1.0))

    nc = bacc.Bacc(None)
    s_in = nc.dram_tensor("s", [B_SH, N], F32, kind="ExternalInput")
    j_in = nc.dram_tensor("J", [N, N], F32, kind="ExternalInput")
    jh_in = nc.dram_tensor("JH", [N, N], BF16, kind="ExternalInput")
    jl_in = nc.dram_tensor("JL", [N, N], BF16, kind="ExternalInput")
    h_in = nc.dram_tensor("h", [N], F32, kind="ExternalInput") if has_h else None
    out = nc.dram_tensor("out", [B_SH, N], F32, kind="ExternalOutput")

    with tile.TileContext(nc) as tc:
        with (
            tc.tile_pool(name="persist", bufs=1) as persist,
            tc.tile_pool(name="stats", bufs=1) as stats,
            tc.tile_pool(name="scratch", bufs=2) as scratch,
            tc.tile_pool(name="spool", bufs=4) as spool,
            tc.tile_pool(name="psum", bufs=8, space="PSUM") as psum,
        ):
            identf = persist.tile([P, P], F32, tag="identf", name="identf")
            make_identity(nc, identf)

            h_bc = None
            if has_h:
                h_bc = persist.tile([P, N], F32, tag="hb", name="hb")
                h_ap = h_in.ap()
                nc.sync.dma_start(
                    out=h_bc,
                    in_=bass.AP(tensor=h_ap.tensor, offset=h_ap.offset,
                                ap=[[0, P], [1, N]]),
                )

            # state for steps >= 2 (sign values; fp32 natural layout + bf16
            # transposed stationary)
            cb = [persist.tile([P, N], F32, tag=f"cb{bt}", name=f"cb{bt}")
                  for bt in range(BT)]
            cTb = [persist.tile([P, B_SH], BF16, tag=f"tb{k}", name=f"tb{k}")
                   for k in range(KO)]

            # ---------------- STEP 1: native fp32, bit-exact ----------------
            # s is streamed from HBM per use instead of held resident.
            with (
                tc.tile_pool(name="s1", bufs=1) as s1,
                tc.tile_pool(name="jpool1", bufs=JPOOL_BUFS) as jpool1,
            ):
                cT = [s1.tile([P, B_SH], F32, tag=f"t{k}", name=f"t{k}")
                      for k in range(KO)]
                mx = [stats.tile([P, 1], F32, tag=f"mx{bt}", name=f"mx{bt}")
                      for bt in range(BT)]
                rS = [stats.tile([P, 1], F32, tag=f"rS{bt}", name=f"rS{bt}")
                      for bt in range(BT)]
                # per-(bt,k) partial row maxes, combined after the stream
                pmx = [[stats.tile([P, 1], F32, tag=f"pm{bt}_{k}",
                                   name=f"pm{bt}_{k}") for k in range(KO)]
                       for bt in range(BT)]

                for n in range(NO):
                    nsl = slice(n * NCHUNK, (n + 1) * NCHUNK)
                    pm_t = [psum.tile([P, NCHUNK], F32, tag="pb", name="pm")
                            for _ in range(BT)]
                    for k in range(KO):
                        if n == 0:
                            # fold transpose production into the first chunk:
                            # PE alternates transpose/matmul, stays saturated
                            for bt in range(BT):
                                ck = spool.tile([P, P], F32, tag="ck",
                                                name="ck")
                                nc.sync.dma_start(
                                    out=ck,
                                    in_=s_in.ap()[bt * P:(bt + 1) * P,
                                                  k * P:(k + 1) * P])
                                ps_t = psum.tile([P, NCHUNK], F32, tag="pb",
                                                 name="ps_t")[:, :P]
                                nc.tensor.transpose(ps_t, ck, identf)
                                nc.vector.tensor_copy(
                                    out=cT[k][:, bt * P:(bt + 1) * P],
                                    in_=ps_t)
                                nc.vector.reduce_max(out=pmx[bt][k], in_=ck,
                                                     axis=X)
                        jt = jpool1.tile([P, NCHUNK], F32, tag="jt", name="jt")
                        nc.sync.dma_start(
                            out=jt, in_=j_in.ap()[k * P:(k + 1) * P, nsl])
                        for bt in range(BT):
                            nc.tensor.matmul(
                                pm_t[bt],
                                cT[k][:, bt * P:(bt + 1) * P],
                                jt,
                                start=(k == 0), stop=(k == KO - 1))

                    if n == 0:
                        # softmax stats: combine maxes (order-exact), then a
                        # second pass over s for the exp-sum.  Emitted after
                        # the first k-loop so these DMAs don't delay J tiles;
                        # results are only needed by the first epilogue.
                        for bt in range(BT):
                            for k in range(1, KO):
                                nc.vector.tensor_max(out=pmx[bt][0],
                                                     in0=pmx[bt][0],
                                                     in1=pmx[bt][k])
                            nc.vector.tensor_copy(out=mx[bt], in_=pmx[bt][0])
                            ssum = stats.tile([P, 1], F32, tag=f"ss{bt}",
                                              name=f"ss{bt}")
                            for nch in range(NO):
                                esl = slice(nch * NCHUNK, (nch + 1) * NCHUNK)
                                cc = spool.tile([P, NCHUNK], F32, tag="cc",
                                                name="cc")
                                nc.sync.dma_start(
                                    out=cc,
                                    in_=s_in.ap()[bt * P:(bt + 1) * P, esl])
                                et = scratch.tile([P, NCHUNK], F32, tag="q",
                                                  name="et")
                                nc.vector.tensor_scalar_sub(out=et, in0=cc,
                                                            scalar1=mx[bt])
                                nc.scalar.activation(out=et, in_=et,
                                                     func=AF.Exp)
                                pk = stats.tile([P, 1], F32,
                                                tag=f"pk{bt}_{nch}",
                                                name=f"pk{bt}_{nch}")
                                nc.vector.reduce_sum(out=pk, in_=et, axis=X)
                                if nch == 0:
                                    nc.vector.tensor_copy(out=ssum, in_=pk)
                                else:
                                    nc.vector.tensor_add(out=ssum, in0=ssum,
                                                         in1=pk)
                            nc.vector.reciprocal(out=rS[bt], in_=ssum)

                    for bt in range(BT):
                        m_sl = pm_t[bt]
                        cc = spool.tile([P, NCHUNK], F32, tag="cc", name="cc")
                        nc.sync.dma_start(
                            out=cc, in_=s_in.ap()[bt * P:(bt + 1) * P, nsl])
                        u = None
                        if has_h:
                            u = scratch.tile([P, NCHUNK], F32, tag="u",
                                             name="u")
                            nc.vector.tensor_add(out=u, in0=m_sl,
                                                 in1=h_bc[:, nsl])
                        q = scratch.tile([P, NCHUNK], F32, tag="q", name="q")
                        nc.vector.tensor_scalar_sub(out=q, in0=cc,
                                                    scalar1=mx[bt])
                        nc.scalar.activation(out=q, in_=q, func=AF.Exp)
                        nc.vector.tensor_scalar_mul(out=q, in0=q,
                                                    scalar1=rS[bt])
                        nc.scalar.mul(out=q, in_=q, mul=float(kappa))
                        uu = scratch.tile([P, NCHUNK], F32, tag="uu", name="uu")
                        if has_h:
                            nc.vector.tensor_add(out=uu, in0=u, in1=q)
                        else:
                            nc.vector.tensor_add(out=uu, in0=m_sl, in1=q)
                        if steps == 1:
                            ot = scratch.tile([P, NCHUNK], F32, tag="ot",
                                              name="ot")
                            nc.scalar.activation(out=ot, in_=uu, func=AF.Sign)
                            nc.sync.dma_start(
                                out=out.ap()[bt * P:(bt + 1) * P, nsl], in_=ot)
                        else:
                            nc.scalar.activation(out=cb[bt][:, nsl], in_=uu,
                                                 func=AF.Sign)

            # ---------------- STEPS 2..n: bf16 2-pass ----------------
            with tc.tile_pool(name="jpool2", bufs=JPOOL_BUFS) as jpool2:
                for si in range(1, steps):
                    last = (si == steps - 1)

                    A_ap = [stats.tile([P, 1], F32, tag=f"A{bt}", name=f"A{bt}")
                            for bt in range(BT)]
                    B_ap = [stats.tile([P, 1], F32, tag=f"B{bt}", name=f"B{bt}")
                            for bt in range(BT)]

                    for n in range(NO):
                        nsl = slice(n * NCHUNK, (n + 1) * NCHUNK)
                        pm_t = [psum.tile([P, NCHUNK], F32, tag="pb", name="pm")
                                for _ in range(BT)]
                        for k in range(KO):
                            if n == 0:
                                # fold state transpose into the first chunk
                                for bt in range(BT):
                                    ps_t = psum.tile([P, NCHUNK], F32,
                                                     tag="pb",
                                                     name="ps_t")[:, :P]
                                    nc.tensor.transpose(
                                        ps_t, cb[bt][:, k * P:(k + 1) * P],
                                        identf)
                                    nc.vector.tensor_copy(
                                        out=cTb[k][:, bt * P:(bt + 1) * P],
                                        in_=ps_t)
                            jh = jpool2.tile([P, NCHUNK], BF16, tag="jb",
                                             name="jh")
                            nc.sync.dma_start(
                                out=jh, in_=jh_in.ap()[k * P:(k + 1) * P, nsl])
                            jl = jpool2.tile([P, NCHUNK], BF16, tag="jb",
                                             name="jl")
                            nc.sync.dma_start(
                                out=jl, in_=jl_in.ap()[k * P:(k + 1) * P, nsl])
                            for bt in range(BT):
                                sl = cTb[k][:, bt * P:(bt + 1) * P]
                                nc.tensor.matmul(pm_t[bt], sl, jh,
                                                 start=(k == 0), stop=False)
                                nc.tensor.matmul(pm_t[bt], sl, jl,
                                                 start=False,
                                                 stop=(k == KO - 1))

                        if n == 0:
                            # linearized-softmax coefficients; needed first by
                            # the n=0 epilogue, so emitted after the k-loop
                            for bt in range(BT):
                                S = stats.tile([P, 1], F32, tag=f"S{bt}",
                                               name=f"S{bt}")
                                nc.vector.reduce_sum(out=S, in_=cb[bt], axis=X)
                                D = stats.tile([P, 1], F32, tag=f"D{bt}",
                                               name=f"D{bt}")
                                nc.vector.tensor_scalar(
                                    out=D, in0=S, scalar1=sinh1,
                                    scalar2=dconst, op0=ALU.mult, op1=ALU.add)
                                rec = stats.tile([P, 1], F32, tag=f"rc{bt}",
                                                 name=f"rc{bt}")
                                nc.vector.reciprocal(out=rec, in_=D)
                                nc.vector.tensor_scalar_mul(
                                    out=A_ap[bt], in0=rec, scalar1=kcosh)
                                nc.vector.tensor_scalar_mul(
                                    out=B_ap[bt], in0=rec, scalar1=ksinh)

                        for bt in range(BT):
                            # u = cb*B + mm;  sign(u + A) — adds the exact
                            # linearization of kappa*softmax(cb) for +-1 state
                            u = scratch.tile([P, NCHUNK], F32, tag="uu",
                                             name="u")
                            nc.vector.scalar_tensor_tensor(
                                out=u, in0=cb[bt][:, nsl], scalar=B_ap[bt],
                                in1=pm_t[bt], op0=ALU.mult, op1=ALU.add)
                            if has_h:
                                nc.vector.tensor_add(out=u, in0=u,
                                                     in1=h_bc[:, nsl])
                            if last:
                                ot = scratch.tile([P, NCHUNK], F32, tag="ot",
                                                  name="ot")
                                nc.scalar.sign(ot, u, bias=A_ap[bt])
                                nc.sync.dma_start(
                                    out=out.ap()[bt * P:(bt + 1) * P, nsl],
                                    in_=ot)
                            else:
                                nc.scalar.sign(cb[bt][:, nsl], u,
                                               bias=A_ap[bt])

    nc.finalize()
    return nc


LAST_RESULTS = None  # BassKernelResults from the most recent kernel() call
LAST_NC = None       # finalized Bass module from the most recent kernel() call


def kernel(s, J, h, kappa, steps):
    import os
    import ml_dtypes
    from concourse.bass_utils import run_bass_kernel_spmd

    s = np.ascontiguousarray(np.asarray(s, dtype=np.float32))
    J = np.asarray(J, dtype=np.float32)
    h = np.asarray(h, dtype=np.float32)
    kappa_f = float(np.asarray(kappa))
    steps_i = int(np.asarray(steps))

    Jsym = np.ascontiguousarray(J + J.T)
    JH = Jsym.astype(ml_dtypes.bfloat16)
    JL = (Jsym - JH.astype(np.float32)).astype(ml_dtypes.bfloat16)
    JH = np.ascontiguousarray(JH)
    JL = np.ascontiguousarray(JL)
    has_h = bool(np.any(h))

    nc = _build(steps_i, kappa_f, has_h)
    global LAST_NC
    LAST_NC = nc

    in_maps = []
    for i in range(N_CORES):
        m = {"s": np.ascontiguousarray(s[i * B_SH:(i + 1) * B_SH]),
             "J": Jsym, "JH": JH, "JL": JL}
        if has_h:
            m["h"] = h
        in_maps.append(m)

    trace = os.environ.get("CAM_TRACE", "") == "1"
    res = run_bass_kernel_spmd(nc, in_maps, core_ids=list(range(N_CORES)),
                               trace=trace)
    global LAST_RESULTS
    LAST_RESULTS = res
    out = np.concatenate([r["out"] for r in res.results], axis=0)
    return out.astype(np.float32, copy=False)


if __name__ == "__main__":
    rng = np.random.default_rng(0)
    s = rng.standard_normal((B, N)).astype(np.float32)
    J0 = (0.01 * rng.standard_normal((N, N))).astype(np.float32)
    J = ((J0 + J0.T) / 2).astype(np.float32)
    out = kernel(s=s, J=J, h=np.zeros(N, np.float32),
                 kappa=np.float32(0.2), steps=3)
    print(out.shape, np.unique(out, return_counts=True))


# revision 13
# speedup vs baseline: 1.4082x; 1.0081x over previous
"""Trainium2 Bass kernel for the CurvedAssociativeMemory fixed-point iteration.

Computes, for `steps` iterations:
    s <- sign(s @ (J + J^T) + h + kappa * softmax(s, axis=-1))

Strategy: data-parallel over batch across 8 NeuronCores (512 rows/core),
J replicated and streamed from HBM each step.

Step 1 is native fp32 (2 HW passes per matmul, H/L split of the fp32
operands) with K accumulated in ascending 128-row chunks in PSUM - this
bit-matches the XLA lowering of the jax reference on this hardware, which
matters because sign() flips amplify ~90x through the remaining steps.

Steps 2..n exploit that the state is exactly {-1,+1} after step 1:
  * the matmul runs as TWO bf16 passes against a J = J_hi + J_lo split
    (round-to-nearest bf16 hi + bf16 residual).  s is exact in bf16, so
    the only deviation from the device's native fp32 matmul is J's
    representation error (~2^-18), measured end-to-end at ~250 flipped
    signs out of 16.7M (rel err ~8e-3, gate 2e-2).  2x fewer PE cycles
    than native fp32 (1 cyc/col bf16 streaming vs 2 cyc/col fp32).
  * softmax(c) for c in {-1,+1} is linear in c:
    kappa*softmax(c) = A[b] + B[b]*c with A = kappa*cosh(1)/D,
    B = kappa*sinh(1)/D, D = 4096*cosh(1) + sinh(1)*sum(c) - no exp and
    no max-reduce, just a row-sum and two fused ops.

The transpose of the state (producing the stationary operand) is folded
into the n=0 chunk of each step's matmul loop so the tensor engine never
drains, and J-tile DMAs are not queued behind bulk state DMAs.
"""

import math

import numpy as np

N = 4096          # feature dim
B = 4096          # total batch
N_CORES = 8
B_SH = B // N_CORES   # 512 batch rows per core
P = 128               # partitions
NCHUNK = 512          # matmul moving free-dim per chunk
KO = N // P           # 32 k-tiles
NO = N // NCHUNK      # 8 n-chunks
BT = B_SH // P        # 4 batch tiles per core

JPOOL_BUFS = 6


def _build(steps: int, kappa: float, has_h: bool):
    import concourse.bass as bass
    import concourse.tile as tile
    import concourse.mybir as mybir
    from concourse import bacc
    from concourse.masks import make_identity

    F32 = mybir.dt.float32
    BF16 = mybir.dt.bfloat16
    AF = mybir.ActivationFunctionType
    ALU = mybir.AluOpType
    X = mybir.AxisListType.X

    kcosh = float(kappa * math.cosh(1.0))
    ksinh = float(kappa * math.sinh(1.0))
    dconst = float(N * math.cosh(1.0))
    sinh1 = float(math.sinh(1.0))

    nc = bacc.Bacc(None)
    s_in = nc.dram_tensor("s", [B_SH, N], F32, kind="ExternalInput")
    j_in = nc.dram_tensor("J", [N, N], F32, kind="ExternalInput")
    jh_in = nc.dram_tensor("JH", [N, N], BF16, kind="ExternalInput")
    jl_in = nc.dram_tensor("JL", [N, N], BF16, kind="ExternalInput")
    h_in = nc.dram_tensor("h", [N], F32, kind="ExternalInput") if has_h else None
    out = nc.dram_tensor("out", [B_SH, N], F32, kind="ExternalOutput")

    with tile.TileContext(nc) as tc:
        with (
            tc.tile_pool(name="persist", bufs=1) as persist,
            tc.tile_pool(name="stats", bufs=1) as stats,
            tc.tile_pool(name="scratch", bufs=2) as scratch,
            tc.tile_pool(name="spool", bufs=4) as spool,
            tc.tile_pool(name="psum", bufs=6, space="PSUM") as psum,
            tc.tile_pool(name="psumt", bufs=2, space="PSUM") as psumt,
        ):
            identf = persist.tile([P, P], F32, tag="identf", name="identf")
            make_identity(nc, identf)
            identb = persist.tile([P, P], BF16, tag="identb", name="identb")
            make_identity(nc, identb)

            h_bc = None
            if has_h:
                h_bc = persist.tile([P, N], F32, tag="hb", name="hb")
                h_ap = h_in.ap()
                nc.sync.dma_start(
                    out=h_bc,
                    in_=bass.AP(tensor=h_ap.tensor, offset=h_ap.offset,
                                ap=[[0, P], [1, N]]),
                )

            # state for steps >= 2: sign values, bf16 (exact for +-1).
            # cT sets are double-buffered so step k+1's transposes (emitted
            # inside step k's chunk loop) never conflict with step k's reads.
            cb = [persist.tile([P, N], BF16, tag=f"cb{bt}", name=f"cb{bt}")
                  for bt in range(BT)]
            cTsets = [
                [persist.tile([P, B_SH], BF16, tag=f"tb{v}_{k}",
                              name=f"tb{v}_{k}") for k in range(KO)]
                for v in range(2)
            ]

            def emit_next_transposes(n, dst_cT):
                # build next step's stationary tiles for k in this chunk's
                # column range, right after the epilogue that produced them
                for k in range(4 * n, 4 * n + 4):
                    for bt in range(BT):
                        ps_t = psumt.tile([P, 2 * NCHUNK], BF16, tag="ptb",
                                          name="ps_t")[:, :P]
                        nc.tensor.transpose(
                            ps_t, cb[bt][:, k * P:(k + 1) * P], identb)
                        nc.vector.tensor_copy(
                            out=dst_cT[k][:, bt * P:(bt + 1) * P], in_=ps_t)

            # ---------------- STEP 1: native fp32, bit-exact ----------------
            # s is streamed from HBM per use instead of held resident.
            with (
                tc.tile_pool(name="s1", bufs=1) as s1,
                tc.tile_pool(name="jpool1", bufs=JPOOL_BUFS) as jpool1,
            ):
                cT = [s1.tile([P, B_SH], F32, tag=f"t{k}", name=f"t{k}")
                      for k in range(KO)]
                mx = [stats.tile([P, 1], F32, tag=f"mx{bt}", name=f"mx{bt}")
                      for bt in range(BT)]
                rS = [stats.tile([P, 1], F32, tag=f"rS{bt}", name=f"rS{bt}")
                      for bt in range(BT)]
                # per-(bt,k) partial row maxes, combined after the stream
                pmx = [[stats.tile([P, 1], F32, tag=f"pm{bt}_{k}",
                                   name=f"pm{bt}_{k}") for k in range(KO)]
                       for bt in range(BT)]

                for n in range(NO):
                    nsl = slice(n * NCHUNK, (n + 1) * NCHUNK)
                    pm_t = [psum.tile([P, NCHUNK], F32, tag="pb", name="pm")
                            for _ in range(BT)]
                    for k in range(KO):
                        if n == 0:
                            # fold transpose production into the first chunk:
                            # PE alternates transpose/matmul, stays saturated
                            for bt in range(BT):
                                ck = spool.tile([P, P], F32, tag="ck",
                                                name="ck")
                                nc.sync.dma_start(
                                    out=ck,
                                    in_=s_in.ap()[bt * P:(bt + 1) * P,
                                                  k * P:(k + 1) * P])
                                ps_t = psum.tile([P, NCHUNK], F32, tag="pb",
                                                 name="ps_t")[:, :P]
                                nc.tensor.transpose(ps_t, ck, identf)
                                nc.vector.tensor_copy(
                                    out=cT[k][:, bt * P:(bt + 1) * P],
                                    in_=ps_t)
                                nc.vector.reduce_max(out=pmx[bt][k], in_=ck,
                                                     axis=X)
                        jt = jpool1.tile([P, NCHUNK], F32, tag="jt", name="jt")
                        nc.sync.dma_start(
                            out=jt, in_=j_in.ap()[k * P:(k + 1) * P, nsl])
                        for bt in range(BT):
                            nc.tensor.matmul(
                                pm_t[bt],
                                cT[k][:, bt * P:(bt + 1) * P],
                                jt,
                                start=(k == 0), stop=(k == KO - 1))

                    if n == 0:
                        # softmax stats: combine maxes (order-exact), then a
                        # second pass over s for the exp-sum.  Emitted after
                        # the first k-loop so these DMAs don't delay J tiles;
                        # results are only needed by the first epilogue.
                        for bt in range(BT):
                            for k in range(1, KO):
                                nc.vector.tensor_max(out=pmx[bt][0],
                                                     in0=pmx[bt][0],
                                                     in1=pmx[bt][k])
                            nc.vector.tensor_copy(out=mx[bt], in_=pmx[bt][0])
                            ssum = stats.tile([P, 1], F32, tag=f"ss{bt}",
                                              name=f"ss{bt}")
                            for nch in range(NO):
                                esl = slice(nch * NCHUNK, (nch + 1) * NCHUNK)
                                cc = spool.tile([P, NCHUNK], F32, tag="cc",
                                                name="cc")
                                nc.sync.dma_start(
                                    out=cc,
                                    in_=s_in.ap()[bt * P:(bt + 1) * P, esl])
                                et = scratch.tile([P, NCHUNK], F32, tag="q",
                                                  name="et")
                                nc.vector.tensor_scalar_sub(out=et, in0=cc,
                                                            scalar1=mx[bt])
                                nc.scalar.activation(out=et, in_=et,
                                                     func=AF.Exp)
                                pk = stats.tile([P, 1], F32,
                                                tag=f"pk{bt}_{nch}",
                                                name=f"pk{bt}_{nch}")
                                nc.vector.reduce_sum(out=pk, in_=et, axis=X)
                                if nch == 0:
                                    nc.vector.tensor_copy(out=ssum, in_=pk)
                                else:
                                    nc.vector.tensor_add(out=ssum, in0=ssum,
                                                         in1=pk)
                            nc.vector.reciprocal(out=rS[bt], in_=ssum)

                    for bt in range(BT):
                        m_sl = pm_t[bt]
                        cc = spool.tile([P, NCHUNK], F32, tag="cc", name="cc")
                        nc.sync.dma_start(
                            out=cc, in_=s_in.ap()[bt * P:(bt + 1) * P, nsl])
                        u = None
                        if has_h:
                            u = scratch.tile([P, NCHUNK], F32, tag="u",
                                             name="u")
                            nc.vector.tensor_add(out=u, in0=m_sl,
                                                 in1=h_bc[:, nsl])
                        q = scratch.tile([P, NCHUNK], F32, tag="q", name="q")
                        nc.vector.tensor_scalar_sub(out=q, in0=cc,
                                                    scalar1=mx[bt])
                        nc.scalar.activation(out=q, in_=q, func=AF.Exp)
                        nc.vector.tensor_scalar_mul(out=q, in0=q,
                                                    scalar1=rS[bt])
                        nc.scalar.mul(out=q, in_=q, mul=float(kappa))
                        uu = scratch.tile([P, NCHUNK], F32, tag="uu", name="uu")
                        if has_h:
                            nc.vector.tensor_add(out=uu, in0=u, in1=q)
                        else:
                            nc.vector.tensor_add(out=uu, in0=m_sl, in1=q)
                        if steps == 1:
                            ot = scratch.tile([P, NCHUNK], F32, tag="ot",
                                              name="ot")
                            nc.scalar.activation(out=ot, in_=uu, func=AF.Sign)
                            nc.sync.dma_start(
                                out=out.ap()[bt * P:(bt + 1) * P, nsl], in_=ot)
                        else:
                            nc.scalar.activation(out=cb[bt][:, nsl], in_=uu,
                                                 func=AF.Sign)
                    if steps > 1:
                        emit_next_transposes(n, cTsets[0])

            # ---------------- STEPS 2..n: bf16 2-pass ----------------
            with tc.tile_pool(name="jpool2", bufs=JPOOL_BUFS) as jpool2:
                for si in range(1, steps):
                    last = (si == steps - 1)
                    cTb = cTsets[(si - 1) % 2]

                    A_ap = [stats.tile([P, 1], F32, tag=f"A{bt}", name=f"A{bt}")
                            for bt in range(BT)]
                    B_ap = [stats.tile([P, 1], F32, tag=f"B{bt}", name=f"B{bt}")
                            for bt in range(BT)]

                    for n in range(NO):
                        nsl = slice(n * NCHUNK, (n + 1) * NCHUNK)
                        pm_t = [psum.tile([P, NCHUNK], F32, tag="pb", name="pm")
                                for _ in range(BT)]
                        for k in range(KO):
                            jh = jpool2.tile([P, NCHUNK], BF16, tag="jb",
                                             name="jh")
                            nc.sync.dma_start(
                                out=jh, in_=jh_in.ap()[k * P:(k + 1) * P, nsl])
                            jl = jpool2.tile([P, NCHUNK], BF16, tag="jb",
                                             name="jl")
                            nc.sync.dma_start(
                                out=jl, in_=jl_in.ap()[k * P:(k + 1) * P, nsl])
                            for bt in range(BT):
                                sl = cTb[k][:, bt * P:(bt + 1) * P]
                                nc.tensor.matmul(pm_t[bt], sl, jh,
                                                 start=(k == 0), stop=False)
                                nc.tensor.matmul(pm_t[bt], sl, jl,
                                                 start=False,
                                                 stop=(k == KO - 1))

                        if n == 0:
                            # linearized-softmax coefficients; needed first by
                            # the n=0 epilogue, so emitted after the k-loop
                            for bt in range(BT):
                                S = stats.tile([P, 1], F32, tag=f"S{bt}",
                                               name=f"S{bt}")
                                nc.vector.reduce_sum(out=S, in_=cb[bt], axis=X)
                                D = stats.tile([P, 1], F32, tag=f"D{bt}",
                                               name=f"D{bt}")
                                nc.vector.tensor_scalar(
                                    out=D, in0=S, scalar1=sinh1,
                                    scalar2=dconst, op0=ALU.mult, op1=ALU.add)
                                rec = stats.tile([P, 1], F32, tag=f"rc{bt}",
                                                 name=f"rc{bt}")
                                nc.vector.reciprocal(out=rec, in_=D)
                                nc.vector.tensor_scalar_mul(
                                    out=A_ap[bt], in0=rec, scalar1=kcosh)
                                nc.vector.tensor_scalar_mul(
                                    out=B_ap[bt], in0=rec, scalar1=ksinh)

                        for bt in range(BT):
                            # u = cb*B + mm;  sign(u + A) — adds the exact
                            # linearization of kappa*softmax(cb) for +-1 state
                            u = scratch.tile([P, NCHUNK], F32, tag="uu",
                                             name="u")
                            nc.vector.scalar_tensor_tensor(
                                out=u, in0=cb[bt][:, nsl], scalar=B_ap[bt],
                                in1=pm_t[bt], op0=ALU.mult, op1=ALU.add)
                            if has_h:
                                nc.vector.tensor_add(out=u, in0=u,
                                                     in1=h_bc[:, nsl])
                            if last:
                                ot = scratch.tile([P, NCHUNK], F32, tag="ot",
                                                  name="ot")
                                nc.scalar.sign(ot, u, bias=A_ap[bt])
                                nc.sync.dma_start(
                                    out=out.ap()[bt * P:(bt + 1) * P, nsl],
                                    in_=ot)
                            else:
                                nc.scalar.sign(cb[bt][:, nsl], u,
                                               bias=A_ap[bt])
                        if not last:
                            emit_next_transposes(n, cTsets[si % 2])

    nc.finalize()
    return nc


LAST_RESULTS = None  # BassKernelResults from the most recent kernel() call
LAST_NC = None       # finalized Bass module from the most recent kernel() call


def kernel(s, J, h, kappa, steps):
    import os
    import ml_dtypes
    from concourse.bass_utils import run_bass_kernel_spmd

    s = np.ascontiguousarray(np.asarray(s, dtype=np.float32))
    J = np.asarray(J, dtype=np.float32)
    h = np.asarray(h, dtype=np.float32)
    kappa_f = float(np.asarray(kappa))
    steps_i = int(np.asarray(steps))

    Jsym = np.ascontiguousarray(J + J.T)
    JH = Jsym.astype(ml_dtypes.bfloat16)
    JL = (Jsym - JH.astype(np.float32)).astype(ml_dtypes.bfloat16)
    JH = np.ascontiguousarray(JH)
    JL = np.ascontiguousarray(JL)
    has_h = bool(np.any(h))

    nc = _build(steps_i, kappa_f, has_h)
    global LAST_NC
    LAST_NC = nc

    in_maps = []
    for i in range(N_CORES):
        m = {"s": np.ascontiguousarray(s[i * B_SH:(i + 1) * B_SH]),
             "J": Jsym, "JH": JH, "JL": JL}
        if has_h:
            m["h"] = h
        in_maps.append(m)

    trace = os.environ.get("CAM_TRACE", "") == "1"
    res = run_bass_kernel_spmd(nc, in_maps, core_ids=list(range(N_CORES)),
                               trace=trace)
    global LAST_RESULTS
    LAST_RESULTS = res
    out = np.concatenate([r["out"] for r in res.results], axis=0)
    return out.astype(np.float32, copy=False)


if __name__ == "__main__":
    rng = np.random.default_rng(0)
    s = rng.standard_normal((B, N)).astype(np.float32)
    J0 = (0.01 * rng.standard_normal((N, N))).astype(np.float32)
    J = ((J0 + J0.T) / 2).astype(np.float32)
    out = kernel(s=s, J=J, h=np.zeros(N, np.float32),
                 kappa=np.float32(0.2), steps=3)
    print(out.shape, np.unique(out, return_counts=True))


# revision 14
# speedup vs baseline: 1.4579x; 1.0353x over previous
"""Trainium2 Bass kernel for the CurvedAssociativeMemory fixed-point iteration.

Computes, for `steps` iterations:
    s <- sign(s @ (J + J^T) + h + kappa * softmax(s, axis=-1))

Strategy: data-parallel over batch across 8 NeuronCores (512 rows/core),
J replicated and streamed from HBM each step.

Step 1 is native fp32 (2 HW passes per matmul, H/L split of the fp32
operands) with K accumulated in ascending 128-row chunks in PSUM - this
bit-matches the XLA lowering of the jax reference on this hardware, which
matters because sign() flips amplify ~90x through the remaining steps.

Steps 2..n exploit that the state is exactly {-1,+1} after step 1:
  * the matmul runs as TWO bf16 passes against a J = J_hi + J_lo split
    (round-to-nearest bf16 hi + bf16 residual).  s is exact in bf16, so
    the only deviation from the device's native fp32 matmul is J's
    representation error (~2^-18), measured end-to-end at ~250 flipped
    signs out of 16.7M (rel err ~8e-3, gate 2e-2).  2x fewer PE cycles
    than native fp32 (1 cyc/col bf16 streaming vs 2 cyc/col fp32).
  * softmax(c) for c in {-1,+1} is linear in c:
    kappa*softmax(c) = A[b] + B[b]*c with A = kappa*cosh(1)/D,
    B = kappa*sinh(1)/D, D = 4096*cosh(1) + sinh(1)*sum(c) - no exp and
    no max-reduce, just a row-sum and two fused ops.

The transpose of the state (producing the stationary operand) is folded
into the n=0 chunk of each step's matmul loop so the tensor engine never
drains, and J-tile DMAs are not queued behind bulk state DMAs.
"""

import math

import numpy as np

N = 4096          # feature dim
B = 4096          # total batch
N_CORES = 8
B_SH = B // N_CORES   # 512 batch rows per core
P = 128               # partitions
NCHUNK = 512          # matmul moving free-dim per chunk
KO = N // P           # 32 k-tiles
NO = N // NCHUNK      # 8 n-chunks
BT = B_SH // P        # 4 batch tiles per core

JPOOL_BUFS = 6


def _build(steps: int, kappa: float, has_h: bool):
    import concourse.bass as bass
    import concourse.tile as tile
    import concourse.mybir as mybir
    from concourse import bacc
    from concourse.masks import make_identity

    F32 = mybir.dt.float32
    BF16 = mybir.dt.bfloat16
    AF = mybir.ActivationFunctionType
    ALU = mybir.AluOpType
    X = mybir.AxisListType.X

    kcosh = float(kappa * math.cosh(1.0))
    ksinh = float(kappa * math.sinh(1.0))
    dconst = float(N * math.cosh(1.0))
    sinh1 = float(math.sinh(1.0))

    nc = bacc.Bacc(None)
    s_in = nc.dram_tensor("s", [B_SH, N], F32, kind="ExternalInput")
    j_in = nc.dram_tensor("J", [N, N], F32, kind="ExternalInput")
    jh_in = nc.dram_tensor("JH", [N, N], BF16, kind="ExternalInput")
    jl_in = nc.dram_tensor("JL", [N, N], BF16, kind="ExternalInput")
    h_in = nc.dram_tensor("h", [N], F32, kind="ExternalInput") if has_h else None
    out = nc.dram_tensor("out", [B_SH, N], F32, kind="ExternalOutput")

    with tile.TileContext(nc) as tc:
        with (
            tc.tile_pool(name="persist", bufs=1) as persist,
            tc.tile_pool(name="stats", bufs=1) as stats,
            tc.tile_pool(name="scratch", bufs=2) as scratch,
            tc.tile_pool(name="spool", bufs=4) as spool,
            tc.tile_pool(name="psum", bufs=6, space="PSUM") as psum,
            tc.tile_pool(name="psumt", bufs=2, space="PSUM") as psumt,
        ):
            identf = persist.tile([P, P], F32, tag="identf", name="identf")
            make_identity(nc, identf)
            identb = persist.tile([P, P], BF16, tag="identb", name="identb")
            make_identity(nc, identb)

            h_bc = None
            if has_h:
                h_bc = persist.tile([P, N], F32, tag="hb", name="hb")
                h_ap = h_in.ap()
                nc.sync.dma_start(
                    out=h_bc,
                    in_=bass.AP(tensor=h_ap.tensor, offset=h_ap.offset,
                                ap=[[0, P], [1, N]]),
                )

            # state for steps >= 2: sign values, bf16 (exact for +-1).
            # cT sets are double-buffered so step k+1's transposes (emitted
            # inside step k's chunk loop) never conflict with step k's reads.
            cb = [persist.tile([P, N], BF16, tag=f"cb{bt}", name=f"cb{bt}")
                  for bt in range(BT)]
            cTsets = [
                [persist.tile([P, B_SH], BF16, tag=f"tb{v}_{k}",
                              name=f"tb{v}_{k}") for k in range(KO)]
                for v in range(2)
            ]

            def emit_next_transposes(n, dst_cT):
                # build next step's stationary tiles for k in this chunk's
                # column range, right after the epilogue that produced them
                for k in range(4 * n, 4 * n + 4):
                    for bt in range(BT):
                        ps_t = psumt.tile([P, 2 * NCHUNK], BF16, tag="ptb",
                                          name="ps_t")[:, :P]
                        nc.tensor.transpose(
                            ps_t, cb[bt][:, k * P:(k + 1) * P], identb)
                        nc.vector.tensor_copy(
                            out=dst_cT[k][:, bt * P:(bt + 1) * P], in_=ps_t)

            # ---------------- STEP 1: native fp32, bit-exact ----------------
            # s is streamed from HBM per use instead of held resident.
            with (
                tc.tile_pool(name="s1", bufs=1) as s1,
                tc.tile_pool(name="jpool1", bufs=JPOOL_BUFS) as jpool1,
            ):
                cT = [s1.tile([P, B_SH], F32, tag=f"t{k}", name=f"t{k}")
                      for k in range(KO)]
                rS = [stats.tile([P, 1], F32, tag=f"rS{bt}", name=f"rS{bt}")
                      for bt in range(BT)]
                ssum = [stats.tile([P, 1], F32, tag=f"ss{bt}", name=f"ss{bt}")
                        for bt in range(BT)]

                # softmax WITHOUT max-subtraction: |s| <= ~5.5 so exp(s) <=
                # ~250, no overflow; exp(s)/sum(exp(s)) equals the reference's
                # stabilized softmax to ~1e-7 relative, far inside the ~1e-4
                # the sign() needs.  This kills an entire 8MB DMA pass and the
                # ordering hazard it created in front of the J stream.
                for n in range(NO):
                    nsl = slice(n * NCHUNK, (n + 1) * NCHUNK)
                    pm_t = [psum.tile([P, NCHUNK], F32, tag="pb", name="pm")
                            for _ in range(BT)]
                    for k in range(KO):
                        if n == 0:
                            # fold transpose production into the first chunk:
                            # PE alternates transpose/matmul, stays saturated
                            for bt in range(BT):
                                ck = spool.tile([P, P], F32, tag="ck",
                                                name="ck")
                                nc.sync.dma_start(
                                    out=ck,
                                    in_=s_in.ap()[bt * P:(bt + 1) * P,
                                                  k * P:(k + 1) * P])
                                ps_t = psum.tile([P, NCHUNK], F32, tag="pb",
                                                 name="ps_t")[:, :P]
                                nc.tensor.transpose(ps_t, ck, identf)
                                nc.vector.tensor_copy(
                                    out=cT[k][:, bt * P:(bt + 1) * P],
                                    in_=ps_t)
                            # one exp-sum unit per k-slot: (bt, chunk) =
                            # divmod(k, NO); DMAs interleave with the J stream
                            sbt, snch = divmod(k, NO)
                            esl = slice(snch * NCHUNK, (snch + 1) * NCHUNK)
                            cc = spool.tile([P, NCHUNK], F32, tag="cc",
                                            name="cc")
                            nc.sync.dma_start(
                                out=cc,
                                in_=s_in.ap()[sbt * P:(sbt + 1) * P, esl])
                            et = scratch.tile([P, NCHUNK], F32, tag="q",
                                              name="et")
                            nc.scalar.activation(out=et, in_=cc, func=AF.Exp)
                            pk = stats.tile([P, 1], F32, tag=f"pk{k}",
                                            name=f"pk{k}")
                            nc.vector.reduce_sum(out=pk, in_=et, axis=X)
                            if snch == 0:
                                nc.vector.tensor_copy(out=ssum[sbt], in_=pk)
                            else:
                                nc.vector.tensor_add(out=ssum[sbt],
                                                     in0=ssum[sbt], in1=pk)
                            if snch == NO - 1:
                                nc.vector.reciprocal(out=rS[sbt],
                                                     in_=ssum[sbt])
                        jt = jpool1.tile([P, NCHUNK], F32, tag="jt", name="jt")
                        nc.sync.dma_start(
                            out=jt, in_=j_in.ap()[k * P:(k + 1) * P, nsl])
                        for bt in range(BT):
                            nc.tensor.matmul(
                                pm_t[bt],
                                cT[k][:, bt * P:(bt + 1) * P],
                                jt,
                                start=(k == 0), stop=(k == KO - 1))

                    for bt in range(BT):
                        m_sl = pm_t[bt]
                        cc = spool.tile([P, NCHUNK], F32, tag="cc", name="cc")
                        nc.sync.dma_start(
                            out=cc, in_=s_in.ap()[bt * P:(bt + 1) * P, nsl])
                        u = None
                        if has_h:
                            u = scratch.tile([P, NCHUNK], F32, tag="u",
                                             name="u")
                            nc.vector.tensor_add(out=u, in0=m_sl,
                                                 in1=h_bc[:, nsl])
                        q = scratch.tile([P, NCHUNK], F32, tag="q", name="q")
                        nc.scalar.activation(out=q, in_=cc, func=AF.Exp)
                        nc.vector.tensor_scalar_mul(out=q, in0=q,
                                                    scalar1=rS[bt])
                        nc.scalar.mul(out=q, in_=q, mul=float(kappa))
                        uu = scratch.tile([P, NCHUNK], F32, tag="uu", name="uu")
                        if has_h:
                            nc.vector.tensor_add(out=uu, in0=u, in1=q)
                        else:
                            nc.vector.tensor_add(out=uu, in0=m_sl, in1=q)
                        if steps == 1:
                            ot = scratch.tile([P, NCHUNK], F32, tag="ot",
                                              name="ot")
                            nc.scalar.activation(out=ot, in_=uu, func=AF.Sign)
                            nc.sync.dma_start(
                                out=out.ap()[bt * P:(bt + 1) * P, nsl], in_=ot)
                        else:
                            nc.scalar.activation(out=cb[bt][:, nsl], in_=uu,
                                                 func=AF.Sign)
                    if steps > 1:
                        emit_next_transposes(n, cTsets[0])

            # ---------------- STEPS 2..n: bf16 2-pass ----------------
            with tc.tile_pool(name="jpool2", bufs=JPOOL_BUFS) as jpool2:
                for si in range(1, steps):
                    last = (si == steps - 1)
                    cTb = cTsets[(si - 1) % 2]

                    A_ap = [stats.tile([P, 1], F32, tag=f"A{bt}", name=f"A{bt}")
                            for bt in range(BT)]
                    B_ap = [stats.tile([P, 1], F32, tag=f"B{bt}", name=f"B{bt}")
                            for bt in range(BT)]

                    for n in range(NO):
                        nsl = slice(n * NCHUNK, (n + 1) * NCHUNK)
                        pm_t = [psum.tile([P, NCHUNK], F32, tag="pb", name="pm")
                                for _ in range(BT)]
                        for k in range(KO):
                            jh = jpool2.tile([P, NCHUNK], BF16, tag="jb",
                                             name="jh")
                            nc.sync.dma_start(
                                out=jh, in_=jh_in.ap()[k * P:(k + 1) * P, nsl])
                            jl = jpool2.tile([P, NCHUNK], BF16, tag="jb",
                                             name="jl")
                            nc.sync.dma_start(
                                out=jl, in_=jl_in.ap()[k * P:(k + 1) * P, nsl])
                            for bt in range(BT):
                                sl = cTb[k][:, bt * P:(bt + 1) * P]
                                nc.tensor.matmul(pm_t[bt], sl, jh,
                                                 start=(k == 0), stop=False)
                                nc.tensor.matmul(pm_t[bt], sl, jl,
                                                 start=False,
                                                 stop=(k == KO - 1))

                        if n == 0:
                            # linearized-softmax coefficients; needed first by
                            # the n=0 epilogue, so emitted after the k-loop
                            for bt in range(BT):
                                S = stats.tile([P, 1], F32, tag=f"S{bt}",
                                               name=f"S{bt}")
                                nc.vector.reduce_sum(out=S, in_=cb[bt], axis=X)
                                D = stats.tile([P, 1], F32, tag=f"D{bt}",
                                               name=f"D{bt}")
                                nc.vector.tensor_scalar(
                                    out=D, in0=S, scalar1=sinh1,
                                    scalar2=dconst, op0=ALU.mult, op1=ALU.add)
                                rec = stats.tile([P, 1], F32, tag=f"rc{bt}",
                                                 name=f"rc{bt}")
                                nc.vector.reciprocal(out=rec, in_=D)
                                nc.vector.tensor_scalar_mul(
                                    out=A_ap[bt], in0=rec, scalar1=kcosh)
                                nc.vector.tensor_scalar_mul(
                                    out=B_ap[bt], in0=rec, scalar1=ksinh)

                        for bt in range(BT):
                            # u = cb*B + mm;  sign(u + A) — adds the exact
                            # linearization of kappa*softmax(cb) for +-1 state
                            u = scratch.tile([P, NCHUNK], F32, tag="uu",
                                             name="u")
                            nc.vector.scalar_tensor_tensor(
                                out=u, in0=cb[bt][:, nsl], scalar=B_ap[bt],
                                in1=pm_t[bt], op0=ALU.mult, op1=ALU.add)
                            if has_h:
                                nc.vector.tensor_add(out=u, in0=u,
                                                     in1=h_bc[:, nsl])
                            if last:
                                ot = scratch.tile([P, NCHUNK], F32, tag="ot",
                                                  name="ot")
                                nc.scalar.sign(ot, u, bias=A_ap[bt])
                                nc.sync.dma_start(
                                    out=out.ap()[bt * P:(bt + 1) * P, nsl],
                                    in_=ot)
                            else:
                                nc.scalar.sign(cb[bt][:, nsl], u,
                                               bias=A_ap[bt])
                        if not last:
                            emit_next_transposes(n, cTsets[si % 2])

    nc.finalize()
    return nc


LAST_RESULTS = None  # BassKernelResults from the most recent kernel() call
LAST_NC = None       # finalized Bass module from the most recent kernel() call


def kernel(s, J, h, kappa, steps):
    import os
    import ml_dtypes
    from concourse.bass_utils import run_bass_kernel_spmd

    s = np.ascontiguousarray(np.asarray(s, dtype=np.float32))
    J = np.asarray(J, dtype=np.float32)
    h = np.asarray(h, dtype=np.float32)
    kappa_f = float(np.asarray(kappa))
    steps_i = int(np.asarray(steps))

    Jsym = np.ascontiguousarray(J + J.T)
    JH = Jsym.astype(ml_dtypes.bfloat16)
    JL = (Jsym - JH.astype(np.float32)).astype(ml_dtypes.bfloat16)
    JH = np.ascontiguousarray(JH)
    JL = np.ascontiguousarray(JL)
    has_h = bool(np.any(h))

    nc = _build(steps_i, kappa_f, has_h)
    global LAST_NC
    LAST_NC = nc

    in_maps = []
    for i in range(N_CORES):
        m = {"s": np.ascontiguousarray(s[i * B_SH:(i + 1) * B_SH]),
             "J": Jsym, "JH": JH, "JL": JL}
        if has_h:
            m["h"] = h
        in_maps.append(m)

    trace = os.environ.get("CAM_TRACE", "") == "1"
    res = run_bass_kernel_spmd(nc, in_maps, core_ids=list(range(N_CORES)),
                               trace=trace)
    global LAST_RESULTS
    LAST_RESULTS = res
    out = np.concatenate([r["out"] for r in res.results], axis=0)
    return out.astype(np.float32, copy=False)


if __name__ == "__main__":
    rng = np.random.default_rng(0)
    s = rng.standard_normal((B, N)).astype(np.float32)
    J0 = (0.01 * rng.standard_normal((N, N))).astype(np.float32)
    J = ((J0 + J0.T) / 2).astype(np.float32)
    out = kernel(s=s, J=J, h=np.zeros(N, np.float32),
                 kappa=np.float32(0.2), steps=3)
    print(out.shape, np.unique(out, return_counts=True))


# revision 16
# speedup vs baseline: 1.4697x; 1.0081x over previous
"""Trainium2 Bass kernel for the CurvedAssociativeMemory fixed-point iteration.

Computes, for `steps` iterations:
    s <- sign(s @ (J + J^T) + h + kappa * softmax(s, axis=-1))

Strategy: data-parallel over batch across 8 NeuronCores (512 rows/core),
J replicated and streamed from HBM each step.

Step 1 is native fp32 (2 HW passes per matmul, H/L split of the fp32
operands) with K accumulated in ascending 128-row chunks in PSUM - this
bit-matches the XLA lowering of the jax reference on this hardware, which
matters because sign() flips amplify ~90x through the remaining steps.

Steps 2..n exploit that the state is exactly {-1,+1} after step 1:
  * the matmul runs as TWO bf16 passes against a J = J_hi + J_lo split
    (round-to-nearest bf16 hi + bf16 residual).  s is exact in bf16, so
    the only deviation from the device's native fp32 matmul is J's
    representation error (~2^-18), measured end-to-end at ~250 flipped
    signs out of 16.7M (rel err ~8e-3, gate 2e-2).  2x fewer PE cycles
    than native fp32 (1 cyc/col bf16 streaming vs 2 cyc/col fp32).
  * softmax(c) for c in {-1,+1} is linear in c:
    kappa*softmax(c) = A[b] + B[b]*c with A = kappa*cosh(1)/D,
    B = kappa*sinh(1)/D, D = 4096*cosh(1) + sinh(1)*sum(c) - no exp and
    no max-reduce, just a row-sum and two fused ops.

The transpose of the state (producing the stationary operand) is folded
into the n=0 chunk of each step's matmul loop so the tensor engine never
drains, and J-tile DMAs are not queued behind bulk state DMAs.
"""

import math

import numpy as np

N = 4096          # feature dim
B = 4096          # total batch
N_CORES = 8
B_SH = B // N_CORES   # 512 batch rows per core
P = 128               # partitions
NCHUNK = 512          # matmul moving free-dim per chunk
KO = N // P           # 32 k-tiles
NO = N // NCHUNK      # 8 n-chunks
BT = B_SH // P        # 4 batch tiles per core

JPOOL1_BUFS = 6
JPOOL2_BUFS = 8


def _build(steps: int, kappa: float, has_h: bool):
    import concourse.bass as bass
    import concourse.tile as tile
    import concourse.mybir as mybir
    from concourse import bacc
    from concourse.masks import make_identity

    F32 = mybir.dt.float32
    BF16 = mybir.dt.bfloat16
    AF = mybir.ActivationFunctionType
    ALU = mybir.AluOpType
    X = mybir.AxisListType.X

    kcosh = float(kappa * math.cosh(1.0))
    ksinh = float(kappa * math.sinh(1.0))
    dconst = float(N * math.cosh(1.0))
    sinh1 = float(math.sinh(1.0))

    nc = bacc.Bacc(None)
    s_in = nc.dram_tensor("s", [B_SH, N], F32, kind="ExternalInput")
    j_in = nc.dram_tensor("J", [N, N], F32, kind="ExternalInput")
    jh_in = nc.dram_tensor("JH", [N, N], BF16, kind="ExternalInput")
    jl_in = nc.dram_tensor("JL", [N, N], BF16, kind="ExternalInput")
    h_in = nc.dram_tensor("h", [N], F32, kind="ExternalInput") if has_h else None
    out = nc.dram_tensor("out", [B_SH, N], F32, kind="ExternalOutput")

    with tile.TileContext(nc) as tc:
        with (
            tc.tile_pool(name="persist", bufs=1) as persist,
            tc.tile_pool(name="stats", bufs=1) as stats,
            tc.tile_pool(name="scratch", bufs=2) as scratch,
            tc.tile_pool(name="spool", bufs=5) as spool,
            tc.tile_pool(name="psum", bufs=6, space="PSUM") as psum,
            tc.tile_pool(name="psumt", bufs=2, space="PSUM") as psumt,
        ):
            identf = persist.tile([P, P], F32, tag="identf", name="identf")
            make_identity(nc, identf)
            identb = persist.tile([P, P], BF16, tag="identb", name="identb")
            make_identity(nc, identb)

            h_bc = None
            if has_h:
                h_bc = persist.tile([P, N], F32, tag="hb", name="hb")
                h_ap = h_in.ap()
                nc.sync.dma_start(
                    out=h_bc,
                    in_=bass.AP(tensor=h_ap.tensor, offset=h_ap.offset,
                                ap=[[0, P], [1, N]]),
                )

            # state for steps >= 2: sign values, bf16 (exact for +-1).
            # cT sets are double-buffered so step k+1's transposes (emitted
            # inside step k's chunk loop) never conflict with step k's reads.
            cb = [persist.tile([P, N], BF16, tag=f"cb{bt}", name=f"cb{bt}")
                  for bt in range(BT)]
            cTsets = [
                [persist.tile([P, B_SH], BF16, tag=f"tb{v}_{k}",
                              name=f"tb{v}_{k}") for k in range(KO)]
                for v in range(2)
            ]

            def emit_next_transposes(n, dst_cT):
                # build next step's stationary tiles for k in this chunk's
                # column range, right after the epilogue that produced them
                for k in range(4 * n, 4 * n + 4):
                    for bt in range(BT):
                        ps_t = psumt.tile([P, 2 * NCHUNK], BF16, tag="ptb",
                                          name="ps_t")[:, :P]
                        nc.tensor.transpose(
                            ps_t, cb[bt][:, k * P:(k + 1) * P], identb)
                        nc.vector.tensor_copy(
                            out=dst_cT[k][:, bt * P:(bt + 1) * P], in_=ps_t)

            # ---------------- STEP 1: native fp32, bit-exact ----------------
            # s is streamed from HBM per use instead of held resident.
            with (
                tc.tile_pool(name="s1", bufs=1) as s1,
                tc.tile_pool(name="jpool1", bufs=JPOOL1_BUFS) as jpool1,
            ):
                cT = [s1.tile([P, B_SH], F32, tag=f"t{k}", name=f"t{k}")
                      for k in range(KO)]
                rS = [stats.tile([P, 1], F32, tag=f"rS{bt}", name=f"rS{bt}")
                      for bt in range(BT)]
                ssum = [stats.tile([P, 1], F32, tag=f"ss{bt}", name=f"ss{bt}")
                        for bt in range(BT)]

                # softmax WITHOUT max-subtraction: |s| <= ~5.5 so exp(s) <=
                # ~250, no overflow; exp(s)/sum(exp(s)) equals the reference's
                # stabilized softmax to ~1e-7 relative, far inside the ~1e-4
                # the sign() needs.  This kills an entire 8MB DMA pass and the
                # ordering hazard it created in front of the J stream.
                for n in range(NO):
                    nsl = slice(n * NCHUNK, (n + 1) * NCHUNK)
                    pm_t = [psum.tile([P, NCHUNK], F32, tag="pb", name="pm")
                            for _ in range(BT)]
                    for k in range(KO):
                        if n == 0:
                            # fold transpose production into the first chunk:
                            # PE alternates transpose/matmul, stays saturated
                            for bt in range(BT):
                                ck = spool.tile([P, P], F32, tag="ck",
                                                name="ck")
                                nc.sync.dma_start(
                                    out=ck,
                                    in_=s_in.ap()[bt * P:(bt + 1) * P,
                                                  k * P:(k + 1) * P])
                                ps_t = psum.tile([P, NCHUNK], F32, tag="pb",
                                                 name="ps_t")[:, :P]
                                nc.tensor.transpose(ps_t, ck, identf)
                                nc.vector.tensor_copy(
                                    out=cT[k][:, bt * P:(bt + 1) * P],
                                    in_=ps_t)
                            # one exp-sum unit per k-slot: (bt, chunk) =
                            # divmod(k, NO); DMAs interleave with the J stream
                            sbt, snch = divmod(k, NO)
                            esl = slice(snch * NCHUNK, (snch + 1) * NCHUNK)
                            cc = spool.tile([P, NCHUNK], F32, tag="cc",
                                            name="cc")
                            nc.sync.dma_start(
                                out=cc,
                                in_=s_in.ap()[sbt * P:(sbt + 1) * P, esl])
                            et = scratch.tile([P, NCHUNK], F32, tag="q",
                                              name="et")
                            nc.scalar.activation(out=et, in_=cc, func=AF.Exp)
                            pk = stats.tile([P, 1], F32, tag=f"pk{k}",
                                            name=f"pk{k}")
                            nc.vector.reduce_sum(out=pk, in_=et, axis=X)
                            if snch == 0:
                                nc.vector.tensor_copy(out=ssum[sbt], in_=pk)
                            else:
                                nc.vector.tensor_add(out=ssum[sbt],
                                                     in0=ssum[sbt], in1=pk)
                            if snch == NO - 1:
                                nc.vector.reciprocal(out=rS[sbt],
                                                     in_=ssum[sbt])
                        jt = jpool1.tile([P, NCHUNK], F32, tag="jt", name="jt")
                        nc.sync.dma_start(
                            out=jt, in_=j_in.ap()[k * P:(k + 1) * P, nsl])
                        for bt in range(BT):
                            nc.tensor.matmul(
                                pm_t[bt],
                                cT[k][:, bt * P:(bt + 1) * P],
                                jt,
                                start=(k == 0), stop=(k == KO - 1))

                    for bt in range(BT):
                        m_sl = pm_t[bt]
                        cc = spool.tile([P, NCHUNK], F32, tag="cc", name="cc")
                        nc.sync.dma_start(
                            out=cc, in_=s_in.ap()[bt * P:(bt + 1) * P, nsl])
                        u = None
                        if has_h:
                            u = scratch.tile([P, NCHUNK], F32, tag="u",
                                             name="u")
                            nc.vector.tensor_add(out=u, in0=m_sl,
                                                 in1=h_bc[:, nsl])
                        q = scratch.tile([P, NCHUNK], F32, tag="q", name="q")
                        nc.scalar.activation(out=q, in_=cc, func=AF.Exp)
                        nc.vector.tensor_scalar_mul(out=q, in0=q,
                                                    scalar1=rS[bt])
                        nc.scalar.mul(out=q, in_=q, mul=float(kappa))
                        uu = scratch.tile([P, NCHUNK], F32, tag="uu", name="uu")
                        if has_h:
                            nc.vector.tensor_add(out=uu, in0=u, in1=q)
                        else:
                            nc.vector.tensor_add(out=uu, in0=m_sl, in1=q)
                        if steps == 1:
                            ot = scratch.tile([P, NCHUNK], F32, tag="ot",
                                              name="ot")
                            nc.scalar.activation(out=ot, in_=uu, func=AF.Sign)
                            nc.sync.dma_start(
                                out=out.ap()[bt * P:(bt + 1) * P, nsl], in_=ot)
                        else:
                            nc.scalar.activation(out=cb[bt][:, nsl], in_=uu,
                                                 func=AF.Sign)
                    if steps > 1:
                        emit_next_transposes(n, cTsets[0])

            # ---------------- STEPS 2..n: bf16 2-pass ----------------
            with tc.tile_pool(name="jpool2", bufs=JPOOL2_BUFS) as jpool2:
                for si in range(1, steps):
                    last = (si == steps - 1)
                    cTb = cTsets[(si - 1) % 2]

                    A_ap = [stats.tile([P, 1], F32, tag=f"A{bt}", name=f"A{bt}")
                            for bt in range(BT)]
                    B_ap = [stats.tile([P, 1], F32, tag=f"B{bt}", name=f"B{bt}")
                            for bt in range(BT)]

                    for n in range(NO):
                        nsl = slice(n * NCHUNK, (n + 1) * NCHUNK)
                        pm_t = [psum.tile([P, NCHUNK], F32, tag="pb", name="pm")
                                for _ in range(BT)]
                        for k in range(KO):
                            jh = jpool2.tile([P, NCHUNK], BF16, tag="jb",
                                             name="jh")
                            nc.sync.dma_start(
                                out=jh, in_=jh_in.ap()[k * P:(k + 1) * P, nsl])
                            jl = jpool2.tile([P, NCHUNK], BF16, tag="jb",
                                             name="jl")
                            nc.sync.dma_start(
                                out=jl, in_=jl_in.ap()[k * P:(k + 1) * P, nsl])
                            for bt in range(BT):
                                sl = cTb[k][:, bt * P:(bt + 1) * P]
                                nc.tensor.matmul(pm_t[bt], sl, jh,
                                                 start=(k == 0), stop=False)
                                nc.tensor.matmul(pm_t[bt], sl, jl,
                                                 start=False,
                                                 stop=(k == KO - 1))

                        if n == 0:
                            # linearized-softmax coefficients; needed first by
                            # the n=0 epilogue, so emitted after the k-loop
                            for bt in range(BT):
                                S = stats.tile([P, 1], F32, tag=f"S{bt}",
                                               name=f"S{bt}")
                                nc.vector.reduce_sum(out=S, in_=cb[bt], axis=X)
                                D = stats.tile([P, 1], F32, tag=f"D{bt}",
                                               name=f"D{bt}")
                                nc.vector.tensor_scalar(
                                    out=D, in0=S, scalar1=sinh1,
                                    scalar2=dconst, op0=ALU.mult, op1=ALU.add)
                                rec = stats.tile([P, 1], F32, tag=f"rc{bt}",
                                                 name=f"rc{bt}")
                                nc.vector.reciprocal(out=rec, in_=D)
                                nc.vector.tensor_scalar_mul(
                                    out=A_ap[bt], in0=rec, scalar1=kcosh)
                                nc.vector.tensor_scalar_mul(
                                    out=B_ap[bt], in0=rec, scalar1=ksinh)

                        for bt in range(BT):
                            # u = cb*B + mm;  sign(u + A) — adds the exact
                            # linearization of kappa*softmax(cb) for +-1 state
                            u = scratch.tile([P, NCHUNK], F32, tag="uu",
                                             name="u")
                            nc.vector.scalar_tensor_tensor(
                                out=u, in0=cb[bt][:, nsl], scalar=B_ap[bt],
                                in1=pm_t[bt], op0=ALU.mult, op1=ALU.add)
                            if has_h:
                                nc.vector.tensor_add(out=u, in0=u,
                                                     in1=h_bc[:, nsl])
                            if last:
                                ot = scratch.tile([P, NCHUNK], F32, tag="ot",
                                                  name="ot")
                                nc.scalar.sign(ot, u, bias=A_ap[bt])
                                nc.sync.dma_start(
                                    out=out.ap()[bt * P:(bt + 1) * P, nsl],
                                    in_=ot)
                            else:
                                nc.scalar.sign(cb[bt][:, nsl], u,
                                               bias=A_ap[bt])
                        if not last:
                            emit_next_transposes(n, cTsets[si % 2])

    nc.finalize()
    return nc


LAST_RESULTS = None  # BassKernelResults from the most recent kernel() call
LAST_NC = None       # finalized Bass module from the most recent kernel() call


def kernel(s, J, h, kappa, steps):
    import os
    import ml_dtypes
    from concourse.bass_utils import run_bass_kernel_spmd

    s = np.ascontiguousarray(np.asarray(s, dtype=np.float32))
    J = np.asarray(J, dtype=np.float32)
    h = np.asarray(h, dtype=np.float32)
    kappa_f = float(np.asarray(kappa))
    steps_i = int(np.asarray(steps))

    Jsym = np.ascontiguousarray(J + J.T)
    JH = Jsym.astype(ml_dtypes.bfloat16)
    JL = (Jsym - JH.astype(np.float32)).astype(ml_dtypes.bfloat16)
    JH = np.ascontiguousarray(JH)
    JL = np.ascontiguousarray(JL)
    has_h = bool(np.any(h))

    nc = _build(steps_i, kappa_f, has_h)
    global LAST_NC
    LAST_NC = nc

    in_maps = []
    for i in range(N_CORES):
        m = {"s": np.ascontiguousarray(s[i * B_SH:(i + 1) * B_SH]),
             "J": Jsym, "JH": JH, "JL": JL}
        if has_h:
            m["h"] = h
        in_maps.append(m)

    trace = os.environ.get("CAM_TRACE", "") == "1"
    res = run_bass_kernel_spmd(nc, in_maps, core_ids=list(range(N_CORES)),
                               trace=trace)
    global LAST_RESULTS
    LAST_RESULTS = res
    out = np.concatenate([r["out"] for r in res.results], axis=0)
    return out.astype(np.float32, copy=False)


if __name__ == "__main__":
    rng = np.random.default_rng(0)
    s = rng.standard_normal((B, N)).astype(np.float32)
    J0 = (0.01 * rng.standard_normal((N, N))).astype(np.float32)
    J = ((J0 + J0.T) / 2).astype(np.float32)
    out = kernel(s=s, J=J, h=np.zeros(N, np.float32),
                 kappa=np.float32(0.2), steps=3)
    print(out.shape, np.unique(out, return_counts=True))


# revision 17
# speedup vs baseline: 1.5106x; 1.0278x over previous
"""Trainium2 Bass kernel for the CurvedAssociativeMemory fixed-point iteration.

Computes, for `steps` iterations:
    s <- sign(s @ (J + J^T) + h + kappa * softmax(s, axis=-1))

Strategy: data-parallel over batch across 8 NeuronCores (512 rows/core),
J replicated and streamed from HBM each step.

Step 1 is native fp32 (2 HW passes per matmul, H/L split of the fp32
operands) with K accumulated in ascending 128-row chunks in PSUM - this
bit-matches the XLA lowering of the jax reference on this hardware, which
matters because sign() flips amplify ~90x through the remaining steps.

Steps 2..n exploit that the state is exactly {-1,+1} after step 1:
  * the matmul runs as TWO bf16 passes against a J = J_hi + J_lo split
    (round-to-nearest bf16 hi + bf16 residual).  s is exact in bf16, so
    the only deviation from the device's native fp32 matmul is J's
    representation error (~2^-18), measured end-to-end at ~250 flipped
    signs out of 16.7M (rel err ~8e-3, gate 2e-2).  2x fewer PE cycles
    than native fp32 (1 cyc/col bf16 streaming vs 2 cyc/col fp32).
  * softmax(c) for c in {-1,+1} is linear in c:
    kappa*softmax(c) = A[b] + B[b]*c with A = kappa*cosh(1)/D,
    B = kappa*sinh(1)/D, D = 4096*cosh(1) + sinh(1)*sum(c) - no exp and
    no max-reduce, just a row-sum and two fused ops.

The transpose of the state (producing the stationary operand) is folded
into the n=0 chunk of each step's matmul loop so the tensor engine never
drains, and J-tile DMAs are not queued behind bulk state DMAs.
"""

import math

import numpy as np

N = 4096          # feature dim
B = 4096          # total batch
N_CORES = 8
B_SH = B // N_CORES   # 512 batch rows per core
P = 128               # partitions
NCHUNK = 512          # matmul moving free-dim per chunk
KO = N // P           # 32 k-tiles
NO = N // NCHUNK      # 8 n-chunks
BT = B_SH // P        # 4 batch tiles per core

JPOOL1_BUFS = 6
JPOOL2_BUFS = 8


def _build(steps: int, kappa: float, has_h: bool):
    import concourse.bass as bass
    import concourse.tile as tile
    import concourse.mybir as mybir
    from concourse import bacc
    from concourse.masks import make_identity

    F32 = mybir.dt.float32
    BF16 = mybir.dt.bfloat16
    AF = mybir.ActivationFunctionType
    ALU = mybir.AluOpType
    X = mybir.AxisListType.X

    kcosh = float(kappa * math.cosh(1.0))
    ksinh = float(kappa * math.sinh(1.0))
    dconst = float(N * math.cosh(1.0))
    sinh1 = float(math.sinh(1.0))

    nc = bacc.Bacc(None)
    s_in = nc.dram_tensor("s", [B_SH, N], F32, kind="ExternalInput")
    j_in = nc.dram_tensor("J", [N, N], F32, kind="ExternalInput")
    jhl_in = nc.dram_tensor("JHL", [N, 2 * N], BF16, kind="ExternalInput")
    h_in = nc.dram_tensor("h", [N], F32, kind="ExternalInput") if has_h else None
    out = nc.dram_tensor("out", [B_SH, N], F32, kind="ExternalOutput")

    with tile.TileContext(nc) as tc:
        with (
            tc.tile_pool(name="persist", bufs=1) as persist,
            tc.tile_pool(name="stats", bufs=1) as stats,
            tc.tile_pool(name="scratch", bufs=2) as scratch,
            tc.tile_pool(name="spool", bufs=5) as spool,
            tc.tile_pool(name="psum", bufs=6, space="PSUM") as psum,
            tc.tile_pool(name="psumt", bufs=2, space="PSUM") as psumt,
        ):
            identf = persist.tile([P, P], F32, tag="identf", name="identf")
            make_identity(nc, identf)
            identb = persist.tile([P, P], BF16, tag="identb", name="identb")
            make_identity(nc, identb)

            h_bc = None
            if has_h:
                h_bc = persist.tile([P, N], F32, tag="hb", name="hb")
                h_ap = h_in.ap()
                nc.sync.dma_start(
                    out=h_bc,
                    in_=bass.AP(tensor=h_ap.tensor, offset=h_ap.offset,
                                ap=[[0, P], [1, N]]),
                )

            # state for steps >= 2: sign values, bf16 (exact for +-1).
            # cT sets are double-buffered so step k+1's transposes (emitted
            # inside step k's chunk loop) never conflict with step k's reads.
            cb = [persist.tile([P, N], BF16, tag=f"cb{bt}", name=f"cb{bt}")
                  for bt in range(BT)]
            cTsets = [
                [persist.tile([P, B_SH], BF16, tag=f"tb{v}_{k}",
                              name=f"tb{v}_{k}") for k in range(KO)]
                for v in range(2)
            ]

            def emit_next_transposes(n, dst_cT):
                # build next step's stationary tiles for k in this chunk's
                # column range, right after the epilogue that produced them
                for k in range(4 * n, 4 * n + 4):
                    for bt in range(BT):
                        ps_t = psumt.tile([P, 2 * NCHUNK], BF16, tag="ptb",
                                          name="ps_t")[:, :P]
                        nc.tensor.transpose(
                            ps_t, cb[bt][:, k * P:(k + 1) * P], identb)
                        nc.vector.tensor_copy(
                            out=dst_cT[k][:, bt * P:(bt + 1) * P], in_=ps_t)

            # ---------------- STEP 1: native fp32, bit-exact ----------------
            # s is streamed from HBM per use instead of held resident.
            with (
                tc.tile_pool(name="s1", bufs=1) as s1,
                tc.tile_pool(name="jpool1", bufs=JPOOL1_BUFS) as jpool1,
            ):
                cT = [s1.tile([P, B_SH], F32, tag=f"t{k}", name=f"t{k}")
                      for k in range(KO)]
                rS = [stats.tile([P, 1], F32, tag=f"rS{bt}", name=f"rS{bt}")
                      for bt in range(BT)]
                ssum = [stats.tile([P, 1], F32, tag=f"ss{bt}", name=f"ss{bt}")
                        for bt in range(BT)]

                # softmax WITHOUT max-subtraction: |s| <= ~5.5 so exp(s) <=
                # ~250, no overflow; exp(s)/sum(exp(s)) equals the reference's
                # stabilized softmax to ~1e-7 relative, far inside the ~1e-4
                # the sign() needs.  This kills an entire 8MB DMA pass and the
                # ordering hazard it created in front of the J stream.
                for n in range(NO):
                    nsl = slice(n * NCHUNK, (n + 1) * NCHUNK)
                    pm_t = [psum.tile([P, NCHUNK], F32, tag="pb", name="pm")
                            for _ in range(BT)]
                    for k in range(KO):
                        if n == 0:
                            # fold transpose production into the first chunk:
                            # PE alternates transpose/matmul, stays saturated
                            for bt in range(BT):
                                ck = spool.tile([P, P], F32, tag="ck",
                                                name="ck")
                                nc.sync.dma_start(
                                    out=ck,
                                    in_=s_in.ap()[bt * P:(bt + 1) * P,
                                                  k * P:(k + 1) * P])
                                ps_t = psum.tile([P, NCHUNK], F32, tag="pb",
                                                 name="ps_t")[:, :P]
                                nc.tensor.transpose(ps_t, ck, identf)
                                nc.vector.tensor_copy(
                                    out=cT[k][:, bt * P:(bt + 1) * P],
                                    in_=ps_t)
                            # one exp-sum unit per k-slot: (bt, chunk) =
                            # divmod(k, NO); DMAs interleave with the J stream
                            sbt, snch = divmod(k, NO)
                            esl = slice(snch * NCHUNK, (snch + 1) * NCHUNK)
                            cc = spool.tile([P, NCHUNK], F32, tag="cc",
                                            name="cc")
                            nc.sync.dma_start(
                                out=cc,
                                in_=s_in.ap()[sbt * P:(sbt + 1) * P, esl])
                            et = scratch.tile([P, NCHUNK], F32, tag="q",
                                              name="et")
                            nc.scalar.activation(out=et, in_=cc, func=AF.Exp)
                            pk = stats.tile([P, 1], F32, tag=f"pk{k}",
                                            name=f"pk{k}")
                            nc.vector.reduce_sum(out=pk, in_=et, axis=X)
                            if snch == 0:
                                nc.vector.tensor_copy(out=ssum[sbt], in_=pk)
                            else:
                                nc.vector.tensor_add(out=ssum[sbt],
                                                     in0=ssum[sbt], in1=pk)
                            if snch == NO - 1:
                                nc.vector.reciprocal(out=rS[sbt],
                                                     in_=ssum[sbt])
                        jt = jpool1.tile([P, NCHUNK], F32, tag="jt", name="jt")
                        nc.sync.dma_start(
                            out=jt, in_=j_in.ap()[k * P:(k + 1) * P, nsl])
                        for bt in range(BT):
                            nc.tensor.matmul(
                                pm_t[bt],
                                cT[k][:, bt * P:(bt + 1) * P],
                                jt,
                                start=(k == 0), stop=(k == KO - 1))

                    for bt in range(BT):
                        m_sl = pm_t[bt]
                        cc = spool.tile([P, NCHUNK], F32, tag="cc", name="cc")
                        nc.sync.dma_start(
                            out=cc, in_=s_in.ap()[bt * P:(bt + 1) * P, nsl])
                        u = None
                        if has_h:
                            u = scratch.tile([P, NCHUNK], F32, tag="u",
                                             name="u")
                            nc.vector.tensor_add(out=u, in0=m_sl,
                                                 in1=h_bc[:, nsl])
                        q = scratch.tile([P, NCHUNK], F32, tag="q", name="q")
                        nc.scalar.activation(out=q, in_=cc, func=AF.Exp)
                        nc.vector.tensor_scalar_mul(out=q, in0=q,
                                                    scalar1=rS[bt])
                        nc.scalar.mul(out=q, in_=q, mul=float(kappa))
                        uu = scratch.tile([P, NCHUNK], F32, tag="uu", name="uu")
                        if has_h:
                            nc.vector.tensor_add(out=uu, in0=u, in1=q)
                        else:
                            nc.vector.tensor_add(out=uu, in0=m_sl, in1=q)
                        if steps == 1:
                            ot = scratch.tile([P, NCHUNK], F32, tag="ot",
                                              name="ot")
                            nc.scalar.activation(out=ot, in_=uu, func=AF.Sign)
                            nc.sync.dma_start(
                                out=out.ap()[bt * P:(bt + 1) * P, nsl], in_=ot)
                        else:
                            nc.scalar.activation(out=cb[bt][:, nsl], in_=uu,
                                                 func=AF.Sign)
                    if steps > 1:
                        emit_next_transposes(n, cTsets[0])

            # ---------------- STEPS 2..n: bf16 2-pass ----------------
            with tc.tile_pool(name="jpool2", bufs=JPOOL2_BUFS) as jpool2:
                for si in range(1, steps):
                    last = (si == steps - 1)
                    cTb = cTsets[(si - 1) % 2]

                    A_ap = [stats.tile([P, 1], F32, tag=f"A{bt}", name=f"A{bt}")
                            for bt in range(BT)]
                    B_ap = [stats.tile([P, 1], F32, tag=f"B{bt}", name=f"B{bt}")
                            for bt in range(BT)]

                    for n in range(NO):
                        nsl = slice(n * NCHUNK, (n + 1) * NCHUNK)
                        pm_t = [psum.tile([P, NCHUNK], F32, tag="pb", name="pm")
                                for _ in range(BT)]
                        for k in range(KO):
                            jhl = jpool2.tile([P, 2 * NCHUNK], BF16, tag="jb",
                                              name="jhl")
                            nc.sync.dma_start(
                                out=jhl,
                                in_=jhl_in.ap()[k * P:(k + 1) * P,
                                                2 * n * NCHUNK:
                                                2 * (n + 1) * NCHUNK])
                            jh = jhl[:, :NCHUNK]
                            jl = jhl[:, NCHUNK:]
                            for bt in range(BT):
                                sl = cTb[k][:, bt * P:(bt + 1) * P]
                                nc.tensor.matmul(pm_t[bt], sl, jh,
                                                 start=(k == 0), stop=False)
                                nc.tensor.matmul(pm_t[bt], sl, jl,
                                                 start=False,
                                                 stop=(k == KO - 1))

                        if n == 0:
                            # linearized-softmax coefficients; needed first by
                            # the n=0 epilogue, so emitted after the k-loop
                            for bt in range(BT):
                                S = stats.tile([P, 1], F32, tag=f"S{bt}",
                                               name=f"S{bt}")
                                nc.vector.reduce_sum(out=S, in_=cb[bt], axis=X)
                                D = stats.tile([P, 1], F32, tag=f"D{bt}",
                                               name=f"D{bt}")
                                nc.vector.tensor_scalar(
                                    out=D, in0=S, scalar1=sinh1,
                                    scalar2=dconst, op0=ALU.mult, op1=ALU.add)
                                rec = stats.tile([P, 1], F32, tag=f"rc{bt}",
                                                 name=f"rc{bt}")
                                nc.vector.reciprocal(out=rec, in_=D)
                                nc.vector.tensor_scalar_mul(
                                    out=A_ap[bt], in0=rec, scalar1=kcosh)
                                nc.vector.tensor_scalar_mul(
                                    out=B_ap[bt], in0=rec, scalar1=ksinh)

                        for bt in range(BT):
                            # u = cb*B + mm;  sign(u + A) — adds the exact
                            # linearization of kappa*softmax(cb) for +-1 state
                            u = scratch.tile([P, NCHUNK], F32, tag="uu",
                                             name="u")
                            nc.vector.scalar_tensor_tensor(
                                out=u, in0=cb[bt][:, nsl], scalar=B_ap[bt],
                                in1=pm_t[bt], op0=ALU.mult, op1=ALU.add)
                            if has_h:
                                nc.vector.tensor_add(out=u, in0=u,
                                                     in1=h_bc[:, nsl])
                            if last:
                                ot = scratch.tile([P, NCHUNK], F32, tag="ot",
                                                  name="ot")
                                nc.scalar.sign(ot, u, bias=A_ap[bt])
                                nc.sync.dma_start(
                                    out=out.ap()[bt * P:(bt + 1) * P, nsl],
                                    in_=ot)
                            else:
                                nc.scalar.sign(cb[bt][:, nsl], u,
                                               bias=A_ap[bt])
                        if not last:
                            emit_next_transposes(n, cTsets[si % 2])

    nc.finalize()
    return nc


LAST_RESULTS = None  # BassKernelResults from the most recent kernel() call
LAST_NC = None       # finalized Bass module from the most recent kernel() call


def kernel(s, J, h, kappa, steps):
    import os
    import ml_dtypes
    from concourse.bass_utils import run_bass_kernel_spmd

    s = np.ascontiguousarray(np.asarray(s, dtype=np.float32))
    J = np.asarray(J, dtype=np.float32)
    h = np.asarray(h, dtype=np.float32)
    kappa_f = float(np.asarray(kappa))
    steps_i = int(np.asarray(steps))

    Jsym = np.ascontiguousarray(J + J.T)
    JH = Jsym.astype(ml_dtypes.bfloat16)
    JL = (Jsym - JH.astype(np.float32)).astype(ml_dtypes.bfloat16)
    # pack [JH_chunk | JL_chunk] per 512-column chunk: one DMA per (k, n)
    JHL = np.empty((N, 2 * N), dtype=ml_dtypes.bfloat16)
    for n in range(NO):
        JHL[:, 2 * n * NCHUNK:(2 * n + 1) * NCHUNK] = \
            JH[:, n * NCHUNK:(n + 1) * NCHUNK]
        JHL[:, (2 * n + 1) * NCHUNK:(2 * n + 2) * NCHUNK] = \
            JL[:, n * NCHUNK:(n + 1) * NCHUNK]
    JHL = np.ascontiguousarray(JHL)
    has_h = bool(np.any(h))

    nc = _build(steps_i, kappa_f, has_h)
    global LAST_NC
    LAST_NC = nc

    in_maps = []
    for i in range(N_CORES):
        m = {"s": np.ascontiguousarray(s[i * B_SH:(i + 1) * B_SH]),
             "J": Jsym, "JHL": JHL}
        if has_h:
            m["h"] = h
        in_maps.append(m)

    trace = os.environ.get("CAM_TRACE", "") == "1"
    res = run_bass_kernel_spmd(nc, in_maps, core_ids=list(range(N_CORES)),
                               trace=trace)
    global LAST_RESULTS
    LAST_RESULTS = res
    out = np.concatenate([r["out"] for r in res.results], axis=0)
    return out.astype(np.float32, copy=False)


if __name__ == "__main__":
    rng = np.random.default_rng(0)
    s = rng.standard_normal((B, N)).astype(np.float32)
    J0 = (0.01 * rng.standard_normal((N, N))).astype(np.float32)
    J = ((J0 + J0.T) / 2).astype(np.float32)
    out = kernel(s=s, J=J, h=np.zeros(N, np.float32),
                 kappa=np.float32(0.2), steps=3)
    print(out.shape, np.unique(out, return_counts=True))
